# revision 20
# baseline (speedup 1.0000x reference)
"""Trainium2 Bass kernel for 2-layer GAT (nn_GAT_86535001080291) — v3.

Strategy (dst-sharded graph parallelism over 8 NeuronCores):
  - Inputs per core: x shard transposed bf16 [64, 12544] (~1.6MB), compact
    per-slot metadata (gather indices [NG,16,C16] int16 and dst-slot ids dsl
    [128, TOTVCH] u8), and the small weights.
  - Each core builds table1 for its OWN node shard from x, then AllGathers
    the full table (bf16 [100352, 128] rows) into Shared DRAM.
  - Edge gathers run on gpsimd dma_gather spread over 4 SWDGE queues (the
    descriptor-generation ucode pins one Q7 core pair per queue, so
    round-robining buckets over queues engages all 8 Q7 cores).
  - v3: the one-hot M0 ([slot,128dst] per 128-edge chunk) and its transpose
    are built INLINE in the layer-1 edge pass (bf16, straight in SBUF via a
    DVE is_equal against a repeating iota + PE transposes), used directly as
    matmul operands, and stored bf16 to DRAM only for layer 2's reload.
    The layer-1 AllGather is triggered straight after the table build, so
    edge gathers start as soon as the collective lands instead of waiting
    for a separate M0-build phase (and the per-group loads no longer queue
    behind 25 groups of M0 stores on the sync DMA queue — stores go out on
    the scalar queue instead).
  - Edge pass per group of GST supertiles: gpsimd.dma_gather of 256B source
    rows from the table, d-expansion via M0T matmuls, leakyrelu + exp on
    DVE/ACT over the whole group, message scaling, and scatter-add via M0
    matmuls accumulating in PSUM (ones-column = softmax denominator).
All host-side preprocessing depends only on edge_index (graph structure).
"""
import math
from dataclasses import dataclass, field

import numpy as np
import ml_dtypes

import concourse.bacc as bacc
import concourse.mybir as mybir
from concourse.tile import TileContext
from concourse.masks import make_identity
from concourse.tile_rust import add_dep_helper
from concourse import library_config

F32 = mybir.dt.float32
BF16 = mybir.dt.bfloat16
I16 = mybir.dt.int16
P = 128


@dataclass
class Cfg:
    N: int = 100000
    E: int = 1600000
    IN: int = 64
    HID: int = 32
    HEADS: int = 2
    OUT: int = 32
    neg: float = 0.2
    CORES: int = 8
    GST: int = 2            # supertiles per group
    GBUF: int = 4           # group-pipeline depth (gather-pool buffers)
    QN: int = 4             # SWDGE queues for dma_gather round-robin
    sim_gelu: bool = False  # kept for compat; unused (HW Gelu LUT)
    no_cc: bool = False     # timing experiment: replace collectives w/ copies

    @property
    def SHARD(self):
        return self.N // self.CORES

    @property
    def ST(self):
        return math.ceil(self.SHARD / P)

    @property
    def SHARD_PAD(self):
        return self.ST * P

    @property
    def NT(self):
        return self.CORES * self.SHARD_PAD

    @property
    def BROWS(self):
        nb = self.NBUCK
        return (self.NT + nb - 1) // nb

    @property
    def NBUCK(self):
        return max(1, math.ceil(self.NT / 25088))


@dataclass
class GroupSched:
    sts: list            # absolute supertile ids
    nch: int = 0         # physical 128-slot chunks in group
    vnch: int = 0        # virtual chunks (one per (phys chunk, supertile) pair)
    c16: int = 0         # idx columns (slots/16)
    gch0: int = 0        # global phys chunk offset of this group
    gvc0: int = 0        # global virtual chunk offset of this group
    calls: list = field(default_factory=list)    # per bucket: (off16, nidx, ch0)
    cellstart: dict = field(default_factory=dict)  # (sti, b) -> slot offset
    cellvc0: dict = field(default_factory=dict)    # (sti, b) -> group-rel vc id
    tpl: list = field(default_factory=list)   # (vc, ph, st_abs, first, last) per phys
    aggl: list = field(default_factory=list)  # (vc, ph, sti, first, last) per st


def build_schedule(cfg, C_sb):
    """Shared (core-independent) static schedule.

    Cells (per supertile, bucket) are sized to the exact max-over-cores edge
    count; slots for a (group, bucket) run are pooled back-to-back and only
    the run is padded to 128 (gather output alignment). 128-slot physical
    chunks may straddle supertile boundaries; such chunks get one VIRTUAL
    one-hot column block per supertile present.
    """
    groups = []
    st = 0
    gch0 = 0
    gvc0 = 0
    while st < cfg.ST:
        sts = list(range(st, min(st + cfg.GST, cfg.ST)))
        g = GroupSched(sts=sts, gch0=gch0, gvc0=gvc0)
        slot = 0
        vcs = []  # (vc_rel, ph_rel, sti, st_abs)
        for b in range(cfg.NBUCK):
            off16 = slot // 16   # bucket runs start 128-aligned
            s0b = slot
            for i, s in enumerate(sts):
                size = int(C_sb[s][b])
                g.cellstart[(i, b)] = slot
                g.cellvc0[(i, b)] = len(vcs)
                if size > 0:
                    ph0 = slot // P
                    ph1 = (slot + size - 1) // P
                    for ph in range(ph0, ph1 + 1):
                        vcs.append((len(vcs), ph, i, s))
                slot += size
            slot = (slot + P - 1) // P * P
            g.calls.append((off16, slot - s0b, s0b // P))
        g.nch = slot // P
        g.c16 = slot // 16
        g.vnch = len(vcs)
        by_phys = {}
        for (vc, ph, i, s) in vcs:
            by_phys.setdefault(ph, []).append(vc)
        for (vc, ph, i, s) in vcs:
            lst = by_phys[ph]
            g.tpl.append((vc, ph, s, vc == lst[0], vc == lst[-1]))
        by_st = {}
        for (vc, ph, i, s) in vcs:
            by_st.setdefault(i, []).append((vc, ph))
        for i, s in enumerate(sts):
            lst = by_st.get(i, [])
            for k, (vc, ph) in enumerate(lst):
                g.aggl.append((vc, ph, i, k == 0, k == len(lst) - 1))
        groups.append(g)
        gch0 += g.nch
        gvc0 += g.vnch
        st += cfg.GST
    return groups


def preprocess(edge_index, cfg):
    """Pure graph preprocessing: per-core gather indices + dst-slot metadata."""
    src = edge_index[0].astype(np.int64)
    dst = edge_index[1].astype(np.int64)
    loops = np.arange(cfg.N, dtype=np.int64)
    src = np.concatenate([src, loops])
    dst = np.concatenate([dst, loops])

    SH, SP, ST, NB, BR = cfg.SHARD, cfg.SHARD_PAD, cfg.ST, cfg.NBUCK, cfg.BROWS

    per_core = []
    cnt = np.zeros((cfg.CORES, ST, NB), dtype=np.int64)
    for r in range(cfg.CORES):
        m = (dst >= r * SH) & (dst < (r + 1) * SH)
        s_r = src[m]
        d_r = dst[m] - r * SH
        srow = (s_r // SH) * SP + (s_r % SH)
        b_r = srow // BR
        st_r = d_r // P
        per_core.append((srow, d_r, b_r, st_r))
        np.add.at(cnt[r], (st_r, b_r), 1)

    C_sb = cnt.max(axis=0)  # [ST, NB] exact max-over-cores cell sizes
    groups = build_schedule(cfg, C_sb)
    NG = len(groups)
    CHmax = max(g.nch for g in groups)
    VCHmax = max(g.vnch for g in groups)
    C16max = max(g.c16 for g in groups)
    C16max += C16max % 2                      # even: idx ships as f32-typed
    TOTVCH = sum(g.vnch for g in groups)
    TOTVCH4 = (TOTVCH + 3) // 4 * 4           # /4: dsl ships as f32-typed

    # lookup tables per (st, b) cell
    gi_tab = np.zeros((ST, NB), np.int64)
    cstart_tab = np.zeros((ST, NB), np.int64)   # group-relative slot offset
    vc0_tab = np.zeros((ST, NB), np.int64)      # GLOBAL vc id of cell's first vc
    ph0_tab = np.zeros((ST, NB), np.int64)      # group-relative phys chunk of it
    for gi, g in enumerate(groups):
        for (i, b), s0 in g.cellstart.items():
            s = g.sts[i]
            gi_tab[s, b] = gi
            cstart_tab[s, b] = s0
            vc0_tab[s, b] = g.gvc0 + g.cellvc0[(i, b)]
            ph0_tab[s, b] = s0 // P

    arrays = []
    for r in range(cfg.CORES):
        srow, d_r, b_r, st_r = per_core[r]
        gi_r = st_r // cfg.GST
        # sort edges by (group, bucket, st), stable
        order = np.lexsort((st_r, b_r, gi_r))
        srow, d_r, b_r, st_r = srow[order], d_r[order], b_r[order], st_r[order]

        # rank within each (st, b) cell
        cell_key = st_r * NB + b_r
        change = np.empty(len(cell_key), dtype=bool)
        change[0] = True
        change[1:] = cell_key[1:] != cell_key[:-1]
        starts = np.flatnonzero(change)
        rank = np.arange(len(cell_key)) - np.repeat(
            starts, np.diff(np.append(starts, len(cell_key))))

        gi_e = gi_tab[st_r, b_r]
        slot = cstart_tab[st_r, b_r] + rank    # group-relative slot
        vcol = vc0_tab[st_r, b_r] + slot // P - ph0_tab[st_r, b_r]

        idx16 = np.zeros((NG, 16, C16max), dtype=np.int16)
        idx16[gi_e, slot % 16, slot // 16] = (srow - b_r * BR).astype(np.int16)

        dsl = np.full((P, TOTVCH4), 255, dtype=np.uint8)
        dsl[slot % P, vcol] = (d_r % P).astype(np.uint8)

        # ship as f32-typed buffers (same bytes) — the tunnel moves f32
        # noticeably faster than 8/16-bit dtypes
        arrays.append(dict(
            idx=np.ascontiguousarray(idx16).view(np.float32),
            dsl=np.ascontiguousarray(dsl).view(np.float32),
        ))

    sched = dict(groups=groups, NG=NG, CHmax=CHmax, VCHmax=VCHmax,
                 C16max=C16max, TOTVCH=TOTVCH, TOTVCH4=TOTVCH4)
    return sched, arrays


def build_nc(cfg, sched):
    nc = bacc.Bacc("TRN2", target_bir_lowering=False,
                   num_swdge_queues=cfg.QN)
    NG, CHmax, C16max = sched["NG"], sched["CHmax"], sched["C16max"]
    VCHmax, TOTVCH, TOTVCH4 = sched["VCHmax"], sched["TOTVCH"], sched["TOTVCH4"]
    groups = sched["groups"]
    ST, NT, SP, NB, BR = cfg.ST, cfg.NT, cfg.SHARD_PAD, cfg.NBUCK, cfg.BROWS
    H = cfg.HEADS

    # ---- external I/O ----
    # All bulk I/O is declared f32-typed (same bytes, device-side bitcast):
    # the host<->device tunnel ships f32 buffers measurably faster.
    xT_ext = nc.dram_tensor("xT", [cfg.IN, SP // 2], F32, kind="ExternalInput")
    W1_ext = nc.dram_tensor("W1", [cfg.IN, H * cfg.HID], F32, kind="ExternalInput")
    as1_ext = nc.dram_tensor("a_src1", [H, cfg.HID], F32, kind="ExternalInput")
    ad1_ext = nc.dram_tensor("a_dst1", [H, cfg.HID], F32, kind="ExternalInput")
    b1_ext = nc.dram_tensor("b1", [1, H * cfg.HID], F32, kind="ExternalInput")
    W2_ext = nc.dram_tensor("W2", [H * cfg.HID, cfg.OUT], F32, kind="ExternalInput")
    as2_ext = nc.dram_tensor("a_src2", [1, cfg.OUT], F32, kind="ExternalInput")
    ad2_ext = nc.dram_tensor("a_dst2", [1, cfg.OUT], F32, kind="ExternalInput")
    b2_ext = nc.dram_tensor("b2", [1, cfg.OUT], F32, kind="ExternalInput")
    idx_ext = nc.dram_tensor("idx", [NG, 16, C16max // 2], F32, kind="ExternalInput")
    dsl_ext = nc.dram_tensor("dsl", [P, TOTVCH4 // 4], F32, kind="ExternalInput")
    out_ext = nc.dram_tensor("out", [cfg.SHARD, cfg.OUT // 2], F32,
                             kind="ExternalOutput")

    with TileContext(nc) as tc:
        with (
            tc.tile_pool(name="dram", bufs=1, space="DRAM") as dpool,
            tc.tile_pool(name="const", bufs=1) as cpool,
            tc.tile_pool(name="work", bufs=3) as wpool,
        ):
            nc.gpsimd.load_library(library_config.mlp)

            t1_shard = dpool.tile([SP, P], BF16)
            t1_full = dpool.tile([NT, P], BF16, addr_space="Shared")
            t2_shard = dpool.tile([SP, P], BF16)
            t2_full = dpool.tile([NT, P], BF16, addr_space="Shared")
            m0_d = dpool.tile([NG, P, VCHmax * P], BF16)
            m0t_d = dpool.tile([NG, P, VCHmax * P], BF16)
            idxr_d = dpool.tile([NG, P, C16max], I16)

            ident = cpool.tile([P, P], F32)
            make_identity(nc, ident[:])
            ident_bf = cpool.tile([P, P], BF16)
            make_identity(nc, ident_bf[:])
            _pp0cm = tc.tile_pool(name="psum0", bufs=2, space="PSUM")
            ppool0 = _pp0cm.__enter__()

            # repeating [0..127] iota, compared against dsl to build one-hots
            iota_rep = cpool.tile([P, VCHmax, P], BF16)
            nc.gpsimd.iota(iota_rep[:], pattern=[[0, VCHmax], [1, P]],
                           base=0, channel_multiplier=0,
                           allow_small_or_imprecise_dtypes=True)

            # ---------- idx replication to 128 partitions (DRAM->DRAM) ----------
            idxr_writes = []
            for r in range(8):
                idxr_writes.append(nc.sync.dma_start(
                    out=idxr_d[:, 16 * r:16 * (r + 1), :],
                    in_=idx_ext[:, :, :].bitcast(I16)).ins)

            # ---------- weight prep ----------
            w1_t = cpool.tile([cfg.IN, H * cfg.HID], F32)
            nc.sync.dma_start(out=w1_t[:], in_=W1_ext[:, :])
            w2_t = cpool.tile([H * cfg.HID, cfg.OUT], F32)
            nc.sync.dma_start(out=w2_t[:], in_=W2_ext[:, :])
            # a vectors as [HID, 1] columns
            av = cpool.tile([cfg.HID, 2 * H + 2], F32)
            for h in range(H):
                nc.sync.dma_start(out=av[:, h:h + 1], in_=as1_ext[h:h + 1, :])
                nc.sync.dma_start(out=av[:, H + h:H + h + 1], in_=ad1_ext[h:h + 1, :])
            nc.sync.dma_start(out=av[:, 2 * H:2 * H + 1], in_=as2_ext[0:1, :])
            nc.sync.dma_start(out=av[:, 2 * H + 1:2 * H + 2], in_=ad2_ext[0:1, :])

            # per-head W1 transposes (base partition 0)
            w1Th = cpool.tile([cfg.HID, H, cfg.IN], F32)
            for h in range(H):
                w1Th_p = ppool0.tile([cfg.HID, cfg.IN], F32, space="PSUM", tag="prep")
                nc.tensor.transpose(out=w1Th_p[:],
                                    in_=w1_t[:, h * cfg.HID:(h + 1) * cfg.HID],
                                    identity=ident[0:cfg.IN, 0:cfg.IN])
                nc.vector.tensor_copy(out=w1Th[:, h, :], in_=w1Th_p[:])
            w2T_p = ppool0.tile([cfg.OUT, H * cfg.HID], F32, space="PSUM", tag="prep")
            nc.tensor.transpose(out=w2T_p[:], in_=w2_t[:, :],
                                identity=ident[0:H * cfg.HID, 0:H * cfg.HID])
            w2T = cpool.tile([cfg.OUT, H * cfg.HID], F32)
            nc.vector.tensor_copy(out=w2T[:], in_=w2T_p[:])

            # logit weight vectors: wv1[:, 0:2H] = per-head [src..., dst...]
            wv_p = ppool0.tile([cfg.IN, 2 * H + 2], F32, space="PSUM", tag="prep2")
            for h in range(H):
                nc.tensor.matmul(out=wv_p[:, h:h + 1],
                                 lhsT=w1Th[:, h, :],
                                 rhs=av[0:cfg.HID, h:h + 1], start=True, stop=True)
                nc.tensor.matmul(out=wv_p[:, H + h:H + h + 1],
                                 lhsT=w1Th[:, h, :],
                                 rhs=av[0:cfg.HID, H + h:H + h + 1], start=True, stop=True)
            # layer2 vectors: W2 @ a_src2 : contraction over OUT
            nc.tensor.matmul(out=wv_p[0:H * cfg.HID, 2 * H:2 * H + 1], lhsT=w2T[:, :],
                             rhs=av[0:cfg.OUT, 2 * H:2 * H + 1], start=True, stop=True)
            nc.tensor.matmul(out=wv_p[0:H * cfg.HID, 2 * H + 1:2 * H + 2], lhsT=w2T[:, :],
                             rhs=av[0:cfg.OUT, 2 * H + 1:2 * H + 2], start=True, stop=True)

            # W1ext bf16 [IN, 70]: [W1h0 | 0 | W1h1 | 0 | s0 s1 d0 d1]
            NC1 = 2 * (cfg.HID + 1) + 2 * H
            SD1 = 2 * (cfg.HID + 1)  # offset of s-cols in table1
            w1e = cpool.tile([cfg.IN, NC1], BF16)
            for h in range(H):
                nc.vector.tensor_copy(out=w1e[:, h * (cfg.HID + 1):h * (cfg.HID + 1) + cfg.HID],
                                      in_=w1_t[:, h * cfg.HID:(h + 1) * cfg.HID])
                nc.vector.memset(w1e[:, h * (cfg.HID + 1) + cfg.HID:(h + 1) * (cfg.HID + 1)], 0.0)
            nc.vector.tensor_copy(out=w1e[:, SD1:SD1 + H], in_=wv_p[:, 0:H])
            nc.vector.tensor_copy(out=w1e[:, SD1 + H:NC1], in_=wv_p[:, H:2 * H])
            # W2ext f32 [64, 34]: [W2 | s2vec | d2vec]
            NC2 = cfg.OUT + 2
            w2e = cpool.tile([H * cfg.HID, NC2], F32)
            nc.vector.tensor_copy(out=w2e[:, 0:cfg.OUT], in_=w2_t[:, :])
            nc.vector.tensor_copy(out=w2e[:, cfg.OUT:NC2],
                                  in_=wv_p[0:H * cfg.HID, 2 * H:2 * H + 2])

            # biases broadcast to all partitions, with a leading unit axis for
            # per-group (GST-wide) broadcasts
            b1_bc = cpool.tile([P, 1, H, cfg.HID], F32)
            b1_row = cpool.tile([1, H * cfg.HID], F32)
            nc.sync.dma_start(out=b1_row[:], in_=b1_ext[:, :])
            nc.gpsimd.partition_broadcast(
                out_ap=b1_bc[:].rearrange("p a h d -> p (a h d)"), in_ap=b1_row[:])
            b2_bc = cpool.tile([P, 1, cfg.OUT], F32)
            b2_row = cpool.tile([1, cfg.OUT], F32)
            nc.sync.dma_start(out=b2_row[:], in_=b2_ext[:, :])
            nc.gpsimd.partition_broadcast(
                out_ap=b2_bc[:].rearrange("p a d -> p (a d)"), in_ap=b2_row[:])

            d1o = cpool.tile([P, ST, H], BF16)
            d2o = cpool.tile([P, ST, 1], BF16)
            g_all = cpool.tile([P, ST, H, cfg.HID], BF16)

            # ---------- phase T1: own-shard table1 build ----------
            t1_writes = []
            for st in range(ST):
                xTt = wpool.tile([cfg.IN, P], BF16, tag="xT")
                nc.sync.dma_start(
                    out=xTt[:],
                    in_=xT_ext[:, st * (P // 2):(st + 1) * (P // 2)].bitcast(BF16))
                hp = ppool0.tile([P, NC1], F32, space="PSUM", tag="hp")
                nc.tensor.matmul(out=hp[:, :], lhsT=xTt[:], rhs=w1e[:, :],
                                 start=True, stop=True)
                pack = wpool.tile([P, P], BF16, tag="pack")
                nc.vector.tensor_copy(out=pack[:, 0:NC1], in_=hp[:, :])
                ones_view = pack[:, 0:SD1].rearrange(
                    "p (h d) -> p h d", h=H)[:, :, cfg.HID:cfg.HID + 1]
                nc.vector.memset(ones_view, 1.0)
                nc.vector.tensor_copy(out=d1o[:, st, :], in_=hp[:, SD1 + H:SD1 + 2 * H])
                t1_writes.append(nc.sync.dma_start(
                    out=t1_shard[st * P:(st + 1) * P, :], in_=pack[:]).ins)

            _pp0cm.__exit__(None, None, None)

            # layer-1 AllGather fires as soon as the shard table is written;
            # edge gathers only wait on this (M0 builds run concurrently).
            if cfg.no_cc:
                ag1 = nc.sync.dma_start(out=t1_full[0:SP, :], in_=t1_shard[:])
            else:
                ag1 = nc.gpsimd.collective_compute(
                    "AllGather", mybir.AluOpType.bypass,
                    ins=[t1_shard[:].opt()], outs=[t1_full[:].opt()],
                    replica_groups=[list(range(cfg.CORES))])
            for w in t1_writes:
                add_dep_helper(ag1.ins, w, reason="t1 shard complete before AG")
            fences = {1: ag1.ins}

            _gpcm = tc.tile_pool(name="gath", bufs=cfg.GBUF)
            gpool = _gpcm.__enter__()
            m0w = {}

            # ---------- shared edge-pass ----------
            def edge_pass(layer):
                # Per-layer PSUM pools: layer 1 needs tps/gT/h2p banks too, so
                # tp/agg stay at depth 2; layer 2 only needs tp/agg and gets
                # depth 4 (8 banks total either way).
                if layer == 1:
                    table, heads, scol = t1_full, H, SD1
                    mw = cfg.HID + 1   # per-head message width (h | ones)
                    down = d1o
                    edepth = 2
                else:
                    table, heads, scol = t2_full, 1, cfg.OUT + 1
                    mw = cfg.OUT + 1
                    down = d2o
                    edepth = 4
                _ppe = tc.tile_pool(name=f"psum_e{layer}", bufs=edepth,
                                    space="PSUM")
                ppool1 = _ppe.__enter__()
                if layer == 1:
                    _ppt = tc.tile_pool(name="psum_t", bufs=1, space="PSUM")
                    ppool2 = _ppt.__enter__()
                for gi, g in enumerate(groups):
                    nch = g.nch
                    vnch = g.vnch
                    L = len(g.sts)
                    # gathers first: independent of the M0 build, they only
                    # need the idx slice and the table fence
                    idx_t = gpool.tile([P, C16max], I16, tag="idx")
                    ld = nc.sync.dma_start(out=idx_t[:, 0:g.c16],
                                           in_=idxr_d[gi, :, 0:g.c16])
                    for w in idxr_writes:
                        add_dep_helper(ld.ins, w, reason="idx replicated")
                    gath = gpool.tile([P, CHmax, P], BF16, tag="gath")
                    for b in range(NB):
                        off16, nidx, ch0 = g.calls[b]
                        while nidx > 0:
                            n = min(nidx, 4096)
                            gi_inst = nc.gpsimd.dma_gather(
                                gath[:, ch0:ch0 + n // P, :],
                                table[b * BR:NT, :],
                                idx_t[:, off16:off16 + n // 16],
                                n, n, P, single_packet=False,
                                queue_num=b % cfg.QN)
                            add_dep_helper(gi_inst.ins, fences[layer],
                                           reason="table ready before gather")
                            nidx -= n
                            ch0 += n // P
                            off16 += n // 16
                    if layer == 1:
                        # M0 / M0T built inline in SBUF (bf16), stored to
                        # DRAM (scalar DMA queue) only for layer-2 reload
                        dslt8 = gpool.tile([P, VCHmax], mybir.dt.uint8, tag="dsl8")
                        nc.sync.dma_start(
                            out=dslt8[:, 0:vnch],
                            in_=dsl_ext[:, :].bitcast(mybir.dt.uint8)
                                [:, g.gvc0:g.gvc0 + vnch])
                        dslt = gpool.tile([P, VCHmax], BF16, tag="dsl")
                        nc.scalar.activation(
                            out=dslt[:, 0:vnch], in_=dslt8[:, 0:vnch],
                            func=mybir.ActivationFunctionType.Copy)
                        m0_t = gpool.tile([P, VCHmax * P], BF16, tag="m0")
                        nc.vector.tensor_tensor(
                            out=m0_t[:, 0:vnch * P].rearrange(
                                "p (a b) -> p a b", b=P),
                            in0=iota_rep[:, 0:vnch, :],
                            in1=dslt[:, 0:vnch].rearrange("p (a b) -> p a b", b=1)
                                .to_broadcast([P, vnch, P]),
                            op=mybir.AluOpType.is_equal)
                        w0 = nc.scalar.dma_start(out=m0_d[gi, :, 0:vnch * P],
                                                 in_=m0_t[:, 0:vnch * P])
                        m0t_t = gpool.tile([P, VCHmax * P], BF16, tag="m0t")
                        for q in range(0, vnch, 4):
                            k = min(4, vnch - q)
                            tps = ppool1.tile([P, 4, P], BF16, space="PSUM",
                                              tag="tps")
                            for j in range(k):
                                nc.tensor.transpose(
                                    out=tps[:, j, :],
                                    in_=m0_t[:, (q + j) * P:(q + j + 1) * P],
                                    identity=ident_bf[:])
                            nc.scalar.activation(
                                out=m0t_t[:, q * P:(q + k) * P],
                                in_=tps[:, 0:k, :].rearrange("p a b -> p (a b)"),
                                func=mybir.ActivationFunctionType.Copy)
                        w1i = nc.scalar.dma_start(out=m0t_d[gi, :, 0:vnch * P],
                                                  in_=m0t_t[:, 0:vnch * P])
                        m0w[gi] = (w0.ins, w1i.ins)
                    else:
                        m0_t = gpool.tile([P, VCHmax * P], BF16, tag="m0")
                        ld = nc.sync.dma_start(out=m0_t[:, 0:vnch * P],
                                               in_=m0_d[gi, :, 0:vnch * P])
                        add_dep_helper(ld.ins, m0w[gi][0], reason="m0 built")
                        m0t_t = gpool.tile([P, VCHmax * P], BF16, tag="m0t")
                        ld = nc.sync.dma_start(out=m0t_t[:, 0:vnch * P],
                                               in_=m0t_d[gi, :, 0:vnch * P])
                        add_dep_helper(ld.ins, m0w[gi][1], reason="m0t built")
                    # d-expansion: tp[pp, ph, h] = d[dslot(pp, ph), h],
                    # accumulated over the phys chunk's virtual columns
                    tp = ppool1.tile([P, CHmax, H], F32, space="PSUM", tag="tp")
                    for (vc, ph, st_abs, first, last) in g.tpl:
                        nc.tensor.matmul(
                            out=tp[:, ph, 0:heads],
                            lhsT=m0t_t[:, vc * P:(vc + 1) * P],
                            rhs=down[:, st_abs, 0:heads],
                            start=first, stop=last)
                    # whole-group softmax numerators: ex = exp(leakyrelu(s + d))
                    ts_t = wpool.tile([P, CHmax, H], F32, tag="ts")
                    ex_t = wpool.tile([P, CHmax, H], F32, tag="ex")
                    nc.vector.tensor_tensor(
                        out=ts_t[:, 0:nch, 0:heads],
                        in0=tp[:, 0:nch, 0:heads],
                        in1=gath[:, 0:nch, scol:scol + heads],
                        op=mybir.AluOpType.add)
                    # exp(leakyrelu(z)) == max(exp(z), exp(neg*z)): two scaled
                    # ACT exps + one DVE max keeps the slope exact (the HW
                    # Lrelu LUT ignores the alpha operand)
                    nc.scalar.activation(
                        out=ex_t[:, 0:nch, 0:heads],
                        in_=ts_t[:, 0:nch, 0:heads],
                        func=mybir.ActivationFunctionType.Exp)
                    nc.scalar.activation(
                        out=ts_t[:, 0:nch, 0:heads],
                        in_=ts_t[:, 0:nch, 0:heads],
                        func=mybir.ActivationFunctionType.Exp, scale=cfg.neg)
                    nc.vector.tensor_tensor(
                        out=ex_t[:, 0:nch, 0:heads],
                        in0=ex_t[:, 0:nch, 0:heads],
                        in1=ts_t[:, 0:nch, 0:heads],
                        op=mybir.AluOpType.max)
                    # scale messages (incl. ones-col -> denominator)
                    for h in range(heads):
                        nc.vector.tensor_tensor(
                            out=gath[:, 0:nch, h * mw:(h + 1) * mw],
                            in0=gath[:, 0:nch, h * mw:(h + 1) * mw],
                            in1=ex_t[:, 0:nch, h:h + 1].to_broadcast([P, nch, mw]),
                            op=mybir.AluOpType.mult)
                    # scatter-add into [dst, heads*mw] PSUM per supertile
                    aggp = ppool1.tile([P, cfg.GST, heads, mw], F32, space="PSUM",
                                       tag="agg")
                    for (vc, ph, sti, first, last) in g.aggl:
                        nc.tensor.matmul(
                            out=aggp[:, sti, :, :].rearrange("p h m -> p (h m)"),
                            lhsT=m0_t[:, vc * P:(vc + 1) * P],
                            rhs=gath[:, ph, 0:heads * mw],
                            start=first, stop=last)
                    # normalize whole group
                    # (layer-1 messages are [h|ones], layer-2 [ones|h])
                    dcol = mw - 1 if layer == 1 else 0
                    rec = wpool.tile([P, cfg.GST, heads, 1], F32, tag="rec")
                    # +eps: pad dst rows have zero denominators (no edges)
                    nc.vector.tensor_scalar_add(
                        out=rec[:, 0:L], in0=aggp[:, 0:L, :, dcol:dcol + 1],
                        scalar1=1e-30)
                    nc.vector.reciprocal(out=rec[:, 0:L], in_=rec[:, 0:L])
                    g0 = g.sts[0]
                    if layer == 1:
                        gv = g_all[:, g0:g0 + L, :, :]
                        nc.vector.tensor_tensor(
                            out=gv, in0=aggp[:, 0:L, :, 0:cfg.HID],
                            in1=rec[:, 0:L].to_broadcast([P, L, heads, cfg.HID]),
                            op=mybir.AluOpType.mult)
                        nc.vector.tensor_tensor(
                            out=gv, in0=gv,
                            in1=b1_bc[:].to_broadcast([P, L, H, cfg.HID]),
                            op=mybir.AluOpType.add)
                        gvf = gv.rearrange("p s h d -> p (s h d)")
                        nc.scalar.activation(
                            out=gvf, in_=gvf,
                            func=mybir.ActivationFunctionType.Gelu)
                        # interleaved table2 build for this group's supertiles
                        for st in g.sts:
                            gT_p = ppool2.tile([H * cfg.HID, P], BF16, space="PSUM",
                                               tag="gT")
                            nc.tensor.transpose(
                                out=gT_p[:],
                                in_=g_all[:, st, :, :].rearrange("p h d -> p (h d)"),
                                identity=ident_bf[:])
                            gT = wpool.tile([H * cfg.HID, P], F32, tag="gTs")
                            nc.scalar.activation(
                                out=gT[:], in_=gT_p[:],
                                func=mybir.ActivationFunctionType.Copy)
                            h2p = ppool2.tile([P, NC2], F32, space="PSUM", tag="h2p")
                            nc.tensor.matmul(out=h2p[:], lhsT=gT[:], rhs=w2e[:, :],
                                             start=True, stop=True)
                            # table-2 row: [ones | h2 | s | d]
                            pack = wpool.tile([P, P], BF16, tag="pack")
                            nc.vector.memset(pack[:, 0:1], 1.0)
                            nc.vector.tensor_copy(out=pack[:, 1:1 + NC2],
                                                  in_=h2p[:, 0:NC2])
                            nc.vector.tensor_copy(out=d2o[:, st, :],
                                                  in_=h2p[:, NC2 - 1:NC2])
                            t2_writes.append(nc.sync.dma_start(
                                out=t2_shard[st * P:(st + 1) * P, :],
                                in_=pack[:]).ins)
                    else:
                        ov = wpool.tile([P, cfg.GST, cfg.OUT], F32, tag="ov")
                        nc.vector.tensor_tensor(
                            out=ov[:, 0:L, :], in0=aggp[:, 0:L, 0, 1:1 + cfg.OUT],
                            in1=rec[:, 0:L, 0, :].to_broadcast([P, L, cfg.OUT]),
                            op=mybir.AluOpType.mult)
                        ovb = wpool.tile([P, cfg.GST, cfg.OUT], BF16, tag="ovb")
                        nc.vector.tensor_tensor(
                            out=ovb[:, 0:L, :], in0=ov[:, 0:L, :],
                            in1=b2_bc[:].to_broadcast([P, L, cfg.OUT]),
                            op=mybir.AluOpType.add)
                        for i, st_abs in enumerate(g.sts):
                            rows = min(P, cfg.SHARD - st_abs * P)
                            nc.sync.dma_start(
                                out=out_ext[st_abs * P:st_abs * P + rows, :]
                                    .bitcast(BF16),
                                in_=ovb[0:rows, i, :])
                if layer == 1:
                    _ppt.__exit__(None, None, None)
                _ppe.__exit__(None, None, None)

            t2_writes = []
            edge_pass(1)

            if cfg.no_cc:
                cc_inst = nc.sync.dma_start(out=t2_full[0:SP, :], in_=t2_shard[:])
            else:
                cc_inst = nc.gpsimd.collective_compute(
                    "AllGather", mybir.AluOpType.bypass,
                    ins=[t2_shard[:].opt()], outs=[t2_full[:].opt()],
                    replica_groups=[list(range(cfg.CORES))])
            for w in t2_writes:
                add_dep_helper(cc_inst.ins, w, reason="t2 shard complete before AG")
            fences[2] = cc_inst.ins

            edge_pass(2)
            _gpcm.__exit__(None, None, None)

    nc.compile()
    return nc


_CACHE = {}


def _get_built(cfg, edge_index):
    key = hash((edge_index.tobytes(), cfg.N, cfg.E, cfg.GST, cfg.sim_gelu,
                cfg.no_cc, cfg.QN, cfg.GBUF))
    if key not in _CACHE:
        sched, arrays = preprocess(edge_index, cfg)
        nc = build_nc(cfg, sched)
        _CACHE[key] = (nc, sched, arrays)
    return _CACHE[key]


def make_in_maps(cfg, arrays, inputs):
    x = np.ascontiguousarray(inputs["x"], dtype=np.float32)
    shared = dict(
        W1=np.ascontiguousarray(inputs["W1"], dtype=np.float32),
        a_src1=np.ascontiguousarray(inputs["a_src1"], dtype=np.float32),
        a_dst1=np.ascontiguousarray(inputs["a_dst1"], dtype=np.float32),
        b1=np.ascontiguousarray(inputs["b1"], dtype=np.float32).reshape(1, -1),
        W2=np.ascontiguousarray(inputs["W2"], dtype=np.float32),
        a_src2=np.ascontiguousarray(inputs["a_src2"], dtype=np.float32),
        a_dst2=np.ascontiguousarray(inputs["a_dst2"], dtype=np.float32),
        b2=np.ascontiguousarray(inputs["b2"], dtype=np.float32).reshape(1, -1),
    )
    in_maps = []
    for r in range(cfg.CORES):
        xr = np.zeros((cfg.SHARD_PAD, cfg.IN), dtype=np.float32)
        xr[0:cfg.SHARD] = x[r * cfg.SHARD:(r + 1) * cfg.SHARD]
        m = dict(shared)
        m["xT"] = np.ascontiguousarray(
            xr.T.astype(ml_dtypes.bfloat16)).view(np.float32)
        m["idx"] = arrays[r]["idx"]
        m["dsl"] = arrays[r]["dsl"]
        in_maps.append(m)
    return in_maps


def kernel(x, edge_index, W1, a_src1, a_dst1, b1, W2, a_src2, a_dst2, b2,
           cfg=None, return_extras=False):
    from concourse.bass_utils import run_bass_kernel_spmd
    cfg = cfg or Cfg()
    nc, sched, arrays = _get_built(cfg, np.asarray(edge_index))
    in_maps = make_in_maps(cfg, arrays, dict(
        x=x, W1=W1, a_src1=a_src1, a_dst1=a_dst1, b1=b1,
        W2=W2, a_src2=a_src2, a_dst2=a_dst2, b2=b2))
    res = run_bass_kernel_spmd(nc, in_maps, list(range(cfg.CORES)))
    out = np.concatenate(
        [np.ascontiguousarray(res.results[r]["out"])
         .view(ml_dtypes.bfloat16).astype(np.float32)
         for r in range(cfg.CORES)],
        axis=0)
    if return_extras:
        return out, res
    return out


# revision 23
# speedup vs baseline: 1.0698x; 1.0698x over previous
"""Trainium2 Bass kernel for 2-layer GAT (nn_GAT_86535001080291) — v3.

Strategy (dst-sharded graph parallelism over 8 NeuronCores):
  - Inputs per core: x shard transposed bf16 [64, 12544] (~1.6MB), compact
    per-slot metadata (gather indices [NG,16,C16] int16 and dst-slot ids dsl
    [128, TOTVCH] u8), and the small weights.
  - Each core builds table1 for its OWN node shard from x, then AllGathers
    the full table (bf16 [100352, 128] rows) into Shared DRAM.
  - Edge gathers run on gpsimd dma_gather spread over 4 SWDGE queues (the
    descriptor-generation ucode pins one Q7 core pair per queue, so
    round-robining buckets over queues engages all 8 Q7 cores).
  - v3: the one-hot M0 ([slot,128dst] per 128-edge chunk) and its transpose
    are built INLINE in the layer-1 edge pass (bf16, straight in SBUF via a
    DVE is_equal against a repeating iota + PE transposes), used directly as
    matmul operands, and stored bf16 to DRAM only for layer 2's reload.
    The layer-1 AllGather is triggered straight after the table build, so
    edge gathers start as soon as the collective lands instead of waiting
    for a separate M0-build phase (and the per-group loads no longer queue
    behind 25 groups of M0 stores on the sync DMA queue — stores go out on
    the scalar queue instead).
  - Edge pass per group of GST supertiles: gpsimd.dma_gather of 256B source
    rows from the table, d-expansion via M0T matmuls, leakyrelu + exp on
    DVE/ACT over the whole group, message scaling, and scatter-add via M0
    matmuls accumulating in PSUM (ones-column = softmax denominator).
All host-side preprocessing depends only on edge_index (graph structure).
"""
import math
from dataclasses import dataclass, field

import numpy as np
import ml_dtypes

import concourse.bacc as bacc
import concourse.mybir as mybir
from concourse.tile import TileContext
from concourse.masks import make_identity
from concourse.tile_rust import add_dep_helper
from concourse import library_config

F32 = mybir.dt.float32
BF16 = mybir.dt.bfloat16
I16 = mybir.dt.int16
P = 128


@dataclass
class Cfg:
    N: int = 100000
    E: int = 1600000
    IN: int = 64
    HID: int = 32
    HEADS: int = 2
    OUT: int = 32
    neg: float = 0.2
    CORES: int = 8
    GST: int = 1            # supertiles per group
    GBUF: int = 8           # group-pipeline depth (gather-pool buffers)
    WBUF: int = 8           # work-pool depth
    QN: int = 4             # SWDGE queues for dma_gather round-robin
    sim_gelu: bool = False  # kept for compat; unused (HW Gelu LUT)
    no_cc: bool = False     # timing experiment: replace collectives w/ copies

    @property
    def SHARD(self):
        return self.N // self.CORES

    @property
    def ST(self):
        return math.ceil(self.SHARD / P)

    @property
    def SHARD_PAD(self):
        return self.ST * P

    @property
    def NT(self):
        return self.CORES * self.SHARD_PAD

    @property
    def BROWS(self):
        nb = self.NBUCK
        return (self.NT + nb - 1) // nb

    @property
    def NBUCK(self):
        return max(1, math.ceil(self.NT / 25088))


@dataclass
class GroupSched:
    sts: list            # absolute supertile ids
    nch: int = 0         # physical 128-slot chunks in group
    vnch: int = 0        # virtual chunks (one per (phys chunk, supertile) pair)
    c16: int = 0         # idx columns (slots/16)
    gch0: int = 0        # global phys chunk offset of this group
    gvc0: int = 0        # global virtual chunk offset of this group
    calls: list = field(default_factory=list)    # per bucket: (off16, nidx, ch0)
    cellstart: dict = field(default_factory=dict)  # (sti, b) -> slot offset
    cellvc0: dict = field(default_factory=dict)    # (sti, b) -> group-rel vc id
    tpl: list = field(default_factory=list)   # (vc, ph, st_abs, first, last) per phys
    aggl: list = field(default_factory=list)  # (vc, ph, sti, first, last) per st


def build_schedule(cfg, C_sb):
    """Shared (core-independent) static schedule.

    Cells (per supertile, bucket) are sized to the exact max-over-cores edge
    count; slots for a (group, bucket) run are pooled back-to-back and only
    the run is padded to 128 (gather output alignment). 128-slot physical
    chunks may straddle supertile boundaries; such chunks get one VIRTUAL
    one-hot column block per supertile present.
    """
    groups = []
    st = 0
    gch0 = 0
    gvc0 = 0
    while st < cfg.ST:
        sts = list(range(st, min(st + cfg.GST, cfg.ST)))
        g = GroupSched(sts=sts, gch0=gch0, gvc0=gvc0)
        slot = 0
        vcs = []  # (vc_rel, ph_rel, sti, st_abs)
        for b in range(cfg.NBUCK):
            off16 = slot // 16   # bucket runs start 128-aligned
            s0b = slot
            for i, s in enumerate(sts):
                size = int(C_sb[s][b])
                g.cellstart[(i, b)] = slot
                g.cellvc0[(i, b)] = len(vcs)
                if size > 0:
                    ph0 = slot // P
                    ph1 = (slot + size - 1) // P
                    for ph in range(ph0, ph1 + 1):
                        vcs.append((len(vcs), ph, i, s))
                slot += size
            slot = (slot + P - 1) // P * P
            g.calls.append((off16, slot - s0b, s0b // P))
        g.nch = slot // P
        g.c16 = slot // 16
        g.vnch = len(vcs)
        by_phys = {}
        for (vc, ph, i, s) in vcs:
            by_phys.setdefault(ph, []).append(vc)
        for (vc, ph, i, s) in vcs:
            lst = by_phys[ph]
            g.tpl.append((vc, ph, s, vc == lst[0], vc == lst[-1]))
        by_st = {}
        for (vc, ph, i, s) in vcs:
            by_st.setdefault(i, []).append((vc, ph))
        for i, s in enumerate(sts):
            lst = by_st.get(i, [])
            for k, (vc, ph) in enumerate(lst):
                g.aggl.append((vc, ph, i, k == 0, k == len(lst) - 1))
        groups.append(g)
        gch0 += g.nch
        gvc0 += g.vnch
        st += cfg.GST
    return groups


def preprocess(edge_index, cfg):
    """Pure graph preprocessing: per-core gather indices + dst-slot metadata."""
    src = edge_index[0].astype(np.int64)
    dst = edge_index[1].astype(np.int64)
    loops = np.arange(cfg.N, dtype=np.int64)
    src = np.concatenate([src, loops])
    dst = np.concatenate([dst, loops])

    SH, SP, ST, NB, BR = cfg.SHARD, cfg.SHARD_PAD, cfg.ST, cfg.NBUCK, cfg.BROWS

    per_core = []
    cnt = np.zeros((cfg.CORES, ST, NB), dtype=np.int64)
    for r in range(cfg.CORES):
        m = (dst >= r * SH) & (dst < (r + 1) * SH)
        s_r = src[m]
        d_r = dst[m] - r * SH
        srow = (s_r // SH) * SP + (s_r % SH)
        b_r = srow // BR
        st_r = d_r // P
        per_core.append((srow, d_r, b_r, st_r))
        np.add.at(cnt[r], (st_r, b_r), 1)

    C_sb = cnt.max(axis=0)  # [ST, NB] exact max-over-cores cell sizes
    groups = build_schedule(cfg, C_sb)
    NG = len(groups)
    CHmax = max(g.nch for g in groups)
    VCHmax = max(g.vnch for g in groups)
    C16max = max(g.c16 for g in groups)
    C16max += C16max % 2                      # even: idx ships as f32-typed
    TOTVCH = sum(g.vnch for g in groups)
    TOTVCH4 = (TOTVCH + 3) // 4 * 4           # /4: dsl ships as f32-typed

    # lookup tables per (st, b) cell
    gi_tab = np.zeros((ST, NB), np.int64)
    cstart_tab = np.zeros((ST, NB), np.int64)   # group-relative slot offset
    vc0_tab = np.zeros((ST, NB), np.int64)      # GLOBAL vc id of cell's first vc
    ph0_tab = np.zeros((ST, NB), np.int64)      # group-relative phys chunk of it
    for gi, g in enumerate(groups):
        for (i, b), s0 in g.cellstart.items():
            s = g.sts[i]
            gi_tab[s, b] = gi
            cstart_tab[s, b] = s0
            vc0_tab[s, b] = g.gvc0 + g.cellvc0[(i, b)]
            ph0_tab[s, b] = s0 // P

    arrays = []
    for r in range(cfg.CORES):
        srow, d_r, b_r, st_r = per_core[r]
        gi_r = st_r // cfg.GST
        # sort edges by (group, bucket, st), stable
        order = np.lexsort((st_r, b_r, gi_r))
        srow, d_r, b_r, st_r = srow[order], d_r[order], b_r[order], st_r[order]

        # rank within each (st, b) cell
        cell_key = st_r * NB + b_r
        change = np.empty(len(cell_key), dtype=bool)
        change[0] = True
        change[1:] = cell_key[1:] != cell_key[:-1]
        starts = np.flatnonzero(change)
        rank = np.arange(len(cell_key)) - np.repeat(
            starts, np.diff(np.append(starts, len(cell_key))))

        gi_e = gi_tab[st_r, b_r]
        slot = cstart_tab[st_r, b_r] + rank    # group-relative slot
        vcol = vc0_tab[st_r, b_r] + slot // P - ph0_tab[st_r, b_r]

        idx16 = np.zeros((NG, 16, C16max), dtype=np.int16)
        idx16[gi_e, slot % 16, slot // 16] = (srow - b_r * BR).astype(np.int16)

        dsl = np.full((P, TOTVCH4), 255, dtype=np.uint8)
        dsl[slot % P, vcol] = (d_r % P).astype(np.uint8)

        # ship as f32-typed buffers (same bytes) — the tunnel moves f32
        # noticeably faster than 8/16-bit dtypes
        arrays.append(dict(
            idx=np.ascontiguousarray(idx16).view(np.float32),
            dsl=np.ascontiguousarray(dsl).view(np.float32),
        ))

    sched = dict(groups=groups, NG=NG, CHmax=CHmax, VCHmax=VCHmax,
                 C16max=C16max, TOTVCH=TOTVCH, TOTVCH4=TOTVCH4)
    return sched, arrays


def build_nc(cfg, sched):
    nc = bacc.Bacc("TRN2", target_bir_lowering=False,
                   num_swdge_queues=cfg.QN)
    NG, CHmax, C16max = sched["NG"], sched["CHmax"], sched["C16max"]
    VCHmax, TOTVCH, TOTVCH4 = sched["VCHmax"], sched["TOTVCH"], sched["TOTVCH4"]
    groups = sched["groups"]
    ST, NT, SP, NB, BR = cfg.ST, cfg.NT, cfg.SHARD_PAD, cfg.NBUCK, cfg.BROWS
    H = cfg.HEADS

    # ---- external I/O ----
    # All bulk I/O is declared f32-typed (same bytes, device-side bitcast):
    # the host<->device tunnel ships f32 buffers measurably faster.
    xT_ext = nc.dram_tensor("xT", [cfg.IN, SP // 2], F32, kind="ExternalInput")
    W1_ext = nc.dram_tensor("W1", [cfg.IN, H * cfg.HID], F32, kind="ExternalInput")
    as1_ext = nc.dram_tensor("a_src1", [H, cfg.HID], F32, kind="ExternalInput")
    ad1_ext = nc.dram_tensor("a_dst1", [H, cfg.HID], F32, kind="ExternalInput")
    b1_ext = nc.dram_tensor("b1", [1, H * cfg.HID], F32, kind="ExternalInput")
    W2_ext = nc.dram_tensor("W2", [H * cfg.HID, cfg.OUT], F32, kind="ExternalInput")
    as2_ext = nc.dram_tensor("a_src2", [1, cfg.OUT], F32, kind="ExternalInput")
    ad2_ext = nc.dram_tensor("a_dst2", [1, cfg.OUT], F32, kind="ExternalInput")
    b2_ext = nc.dram_tensor("b2", [1, cfg.OUT], F32, kind="ExternalInput")
    idx_ext = nc.dram_tensor("idx", [NG, 16, C16max // 2], F32, kind="ExternalInput")
    dsl_ext = nc.dram_tensor("dsl", [P, TOTVCH4 // 4], F32, kind="ExternalInput")
    out_ext = nc.dram_tensor("out", [cfg.SHARD, cfg.OUT // 2], F32,
                             kind="ExternalOutput")

    with TileContext(nc) as tc:
        with (
            tc.tile_pool(name="dram", bufs=1, space="DRAM") as dpool,
            tc.tile_pool(name="const", bufs=1) as cpool,
            tc.tile_pool(name="work", bufs=cfg.WBUF) as wpool,
        ):
            nc.gpsimd.load_library(library_config.mlp)

            t1_shard = dpool.tile([SP, P], BF16)
            t1_full = dpool.tile([NT, P], BF16, addr_space="Shared")
            t2_shard = dpool.tile([SP, P], BF16)
            t2_full = dpool.tile([NT, P], BF16, addr_space="Shared")
            m0_d = dpool.tile([NG, P, VCHmax * P], BF16)
            m0t_d = dpool.tile([NG, P, VCHmax * P], BF16)
            idxr_d = dpool.tile([NG, P, C16max], I16)

            ident = cpool.tile([P, P], F32)
            make_identity(nc, ident[:])
            ident_bf = cpool.tile([P, P], BF16)
            make_identity(nc, ident_bf[:])
            _pp0cm = tc.tile_pool(name="psum0", bufs=2, space="PSUM")
            ppool0 = _pp0cm.__enter__()

            # repeating [0..127] iota, compared against dsl to build one-hots
            iota_rep = cpool.tile([P, VCHmax, P], BF16)
            nc.gpsimd.iota(iota_rep[:], pattern=[[0, VCHmax], [1, P]],
                           base=0, channel_multiplier=0,
                           allow_small_or_imprecise_dtypes=True)

            # ---------- idx replication to 128 partitions (DRAM->DRAM) ----------
            idxr_writes = []
            for r in range(8):
                idxr_writes.append(nc.sync.dma_start(
                    out=idxr_d[:, 16 * r:16 * (r + 1), :],
                    in_=idx_ext[:, :, :].bitcast(I16)).ins)

            # ---------- weight prep ----------
            w1_t = cpool.tile([cfg.IN, H * cfg.HID], F32)
            nc.sync.dma_start(out=w1_t[:], in_=W1_ext[:, :])
            w2_t = cpool.tile([H * cfg.HID, cfg.OUT], F32)
            nc.sync.dma_start(out=w2_t[:], in_=W2_ext[:, :])
            # a vectors as [HID, 1] columns
            av = cpool.tile([cfg.HID, 2 * H + 2], F32)
            for h in range(H):
                nc.sync.dma_start(out=av[:, h:h + 1], in_=as1_ext[h:h + 1, :])
                nc.sync.dma_start(out=av[:, H + h:H + h + 1], in_=ad1_ext[h:h + 1, :])
            nc.sync.dma_start(out=av[:, 2 * H:2 * H + 1], in_=as2_ext[0:1, :])
            nc.sync.dma_start(out=av[:, 2 * H + 1:2 * H + 2], in_=ad2_ext[0:1, :])

            # per-head W1 transposes (base partition 0)
            w1Th = cpool.tile([cfg.HID, H, cfg.IN], F32)
            for h in range(H):
                w1Th_p = ppool0.tile([cfg.HID, cfg.IN], F32, space="PSUM", tag="prep")
                nc.tensor.transpose(out=w1Th_p[:],
                                    in_=w1_t[:, h * cfg.HID:(h + 1) * cfg.HID],
                                    identity=ident[0:cfg.IN, 0:cfg.IN])
                nc.vector.tensor_copy(out=w1Th[:, h, :], in_=w1Th_p[:])
            w2T_p = ppool0.tile([cfg.OUT, H * cfg.HID], F32, space="PSUM", tag="prep")
            nc.tensor.transpose(out=w2T_p[:], in_=w2_t[:, :],
                                identity=ident[0:H * cfg.HID, 0:H * cfg.HID])
            w2T = cpool.tile([cfg.OUT, H * cfg.HID], F32)
            nc.vector.tensor_copy(out=w2T[:], in_=w2T_p[:])

            # logit weight vectors: wv1[:, 0:2H] = per-head [src..., dst...]
            wv_p = ppool0.tile([cfg.IN, 2 * H + 2], F32, space="PSUM", tag="prep2")
            for h in range(H):
                nc.tensor.matmul(out=wv_p[:, h:h + 1],
                                 lhsT=w1Th[:, h, :],
                                 rhs=av[0:cfg.HID, h:h + 1], start=True, stop=True)
                nc.tensor.matmul(out=wv_p[:, H + h:H + h + 1],
                                 lhsT=w1Th[:, h, :],
                                 rhs=av[0:cfg.HID, H + h:H + h + 1], start=True, stop=True)
            # layer2 vectors: W2 @ a_src2 : contraction over OUT
            nc.tensor.matmul(out=wv_p[0:H * cfg.HID, 2 * H:2 * H + 1], lhsT=w2T[:, :],
                             rhs=av[0:cfg.OUT, 2 * H:2 * H + 1], start=True, stop=True)
            nc.tensor.matmul(out=wv_p[0:H * cfg.HID, 2 * H + 1:2 * H + 2], lhsT=w2T[:, :],
                             rhs=av[0:cfg.OUT, 2 * H + 1:2 * H + 2], start=True, stop=True)

            # W1ext bf16 [IN, 70]: [W1h0 | 0 | W1h1 | 0 | s0 s1 d0 d1]
            NC1 = 2 * (cfg.HID + 1) + 2 * H
            SD1 = 2 * (cfg.HID + 1)  # offset of s-cols in table1
            w1e = cpool.tile([cfg.IN, NC1], BF16)
            for h in range(H):
                nc.vector.tensor_copy(out=w1e[:, h * (cfg.HID + 1):h * (cfg.HID + 1) + cfg.HID],
                                      in_=w1_t[:, h * cfg.HID:(h + 1) * cfg.HID])
                nc.vector.memset(w1e[:, h * (cfg.HID + 1) + cfg.HID:(h + 1) * (cfg.HID + 1)], 0.0)
            nc.vector.tensor_copy(out=w1e[:, SD1:SD1 + H], in_=wv_p[:, 0:H])
            nc.vector.tensor_copy(out=w1e[:, SD1 + H:NC1], in_=wv_p[:, H:2 * H])
            # W2ext f32 [64, 34]: [W2 | s2vec | d2vec]
            NC2 = cfg.OUT + 2
            w2e = cpool.tile([H * cfg.HID, NC2], F32)
            nc.vector.tensor_copy(out=w2e[:, 0:cfg.OUT], in_=w2_t[:, :])
            nc.vector.tensor_copy(out=w2e[:, cfg.OUT:NC2],
                                  in_=wv_p[0:H * cfg.HID, 2 * H:2 * H + 2])

            # biases broadcast to all partitions, with a leading unit axis for
            # per-group (GST-wide) broadcasts
            b1_bc = cpool.tile([P, 1, H, cfg.HID], F32)
            b1_row = cpool.tile([1, H * cfg.HID], F32)
            nc.sync.dma_start(out=b1_row[:], in_=b1_ext[:, :])
            nc.gpsimd.partition_broadcast(
                out_ap=b1_bc[:].rearrange("p a h d -> p (a h d)"), in_ap=b1_row[:])
            b2_bc = cpool.tile([P, 1, cfg.OUT], F32)
            b2_row = cpool.tile([1, cfg.OUT], F32)
            nc.sync.dma_start(out=b2_row[:], in_=b2_ext[:, :])
            nc.gpsimd.partition_broadcast(
                out_ap=b2_bc[:].rearrange("p a d -> p (a d)"), in_ap=b2_row[:])

            d1o = cpool.tile([P, ST, H], BF16)
            d2o = cpool.tile([P, ST, 1], BF16)
            g_all = cpool.tile([P, ST, H, cfg.HID], BF16)

            # ---------- phase T1: own-shard table1 build ----------
            t1_writes = []
            for st in range(ST):
                xTt = wpool.tile([cfg.IN, P], BF16, tag="xT")
                nc.sync.dma_start(
                    out=xTt[:],
                    in_=xT_ext[:, st * (P // 2):(st + 1) * (P // 2)].bitcast(BF16))
                hp = ppool0.tile([P, NC1], F32, space="PSUM", tag="hp")
                nc.tensor.matmul(out=hp[:, :], lhsT=xTt[:], rhs=w1e[:, :],
                                 start=True, stop=True)
                pack = wpool.tile([P, P], BF16, tag="pack")
                nc.vector.tensor_copy(out=pack[:, 0:NC1], in_=hp[:, :])
                ones_view = pack[:, 0:SD1].rearrange(
                    "p (h d) -> p h d", h=H)[:, :, cfg.HID:cfg.HID + 1]
                nc.vector.memset(ones_view, 1.0)
                nc.vector.tensor_copy(out=d1o[:, st, :], in_=hp[:, SD1 + H:SD1 + 2 * H])
                t1_writes.append(nc.sync.dma_start(
                    out=t1_shard[st * P:(st + 1) * P, :], in_=pack[:]).ins)

            _pp0cm.__exit__(None, None, None)

            # layer-1 AllGather fires as soon as the shard table is written;
            # edge gathers only wait on this (M0 builds run concurrently).
            if cfg.no_cc:
                ag1 = nc.sync.dma_start(out=t1_full[0:SP, :], in_=t1_shard[:])
            else:
                ag1 = nc.gpsimd.collective_compute(
                    "AllGather", mybir.AluOpType.bypass,
                    ins=[t1_shard[:].opt()], outs=[t1_full[:].opt()],
                    replica_groups=[list(range(cfg.CORES))])
            for w in t1_writes:
                add_dep_helper(ag1.ins, w, reason="t1 shard complete before AG")
            fences = {1: ag1.ins}

            _gpcm = tc.tile_pool(name="gath", bufs=cfg.GBUF)
            gpool = _gpcm.__enter__()
            m0w = {}

            # ---------- shared edge-pass ----------
            def edge_pass(layer):
                # Per-layer PSUM pools: layer 1 needs tps/gT/h2p banks too, so
                # tp/agg stay at depth 2; layer 2 only needs tp/agg and gets
                # depth 4 (8 banks total either way).
                if layer == 1:
                    table, heads, scol = t1_full, H, SD1
                    mw = cfg.HID + 1   # per-head message width (h | ones)
                    down = d1o
                    edepth = 2
                else:
                    table, heads, scol = t2_full, 1, cfg.OUT + 1
                    mw = cfg.OUT + 1
                    down = d2o
                    edepth = 4
                _ppe = tc.tile_pool(name=f"psum_e{layer}", bufs=edepth,
                                    space="PSUM")
                ppool1 = _ppe.__enter__()
                if layer == 1:
                    _ppt = tc.tile_pool(name="psum_t", bufs=1, space="PSUM")
                    ppool2 = _ppt.__enter__()
                for gi, g in enumerate(groups):
                    nch = g.nch
                    vnch = g.vnch
                    L = len(g.sts)
                    # gathers first: independent of the M0 build, they only
                    # need the idx slice and the table fence
                    idx_t = gpool.tile([P, C16max], I16, tag="idx")
                    ld = nc.sync.dma_start(out=idx_t[:, 0:g.c16],
                                           in_=idxr_d[gi, :, 0:g.c16])
                    for w in idxr_writes:
                        add_dep_helper(ld.ins, w, reason="idx replicated")
                    gath = gpool.tile([P, CHmax, P], BF16, tag="gath")
                    for b in range(NB):
                        off16, nidx, ch0 = g.calls[b]
                        while nidx > 0:
                            n = min(nidx, 4096)
                            gi_inst = nc.gpsimd.dma_gather(
                                gath[:, ch0:ch0 + n // P, :],
                                table[b * BR:NT, :],
                                idx_t[:, off16:off16 + n // 16],
                                n, n, P, single_packet=False,
                                queue_num=b % cfg.QN)
                            add_dep_helper(gi_inst.ins, fences[layer],
                                           reason="table ready before gather")
                            nidx -= n
                            ch0 += n // P
                            off16 += n // 16
                    if layer == 1:
                        # M0 / M0T built inline in SBUF (bf16), stored to
                        # DRAM (scalar DMA queue) only for layer-2 reload
                        dslt8 = gpool.tile([P, VCHmax], mybir.dt.uint8, tag="dsl8")
                        nc.sync.dma_start(
                            out=dslt8[:, 0:vnch],
                            in_=dsl_ext[:, :].bitcast(mybir.dt.uint8)
                                [:, g.gvc0:g.gvc0 + vnch])
                        dslt = gpool.tile([P, VCHmax], BF16, tag="dsl")
                        nc.scalar.activation(
                            out=dslt[:, 0:vnch], in_=dslt8[:, 0:vnch],
                            func=mybir.ActivationFunctionType.Copy)
                        m0_t = gpool.tile([P, VCHmax * P], BF16, tag="m0")
                        nc.vector.tensor_tensor(
                            out=m0_t[:, 0:vnch * P].rearrange(
                                "p (a b) -> p a b", b=P),
                            in0=iota_rep[:, 0:vnch, :],
                            in1=dslt[:, 0:vnch].rearrange("p (a b) -> p a b", b=1)
                                .to_broadcast([P, vnch, P]),
                            op=mybir.AluOpType.is_equal)
                        w0 = nc.scalar.dma_start(out=m0_d[gi, :, 0:vnch * P],
                                                 in_=m0_t[:, 0:vnch * P])
                        m0t_t = gpool.tile([P, VCHmax * P], BF16, tag="m0t")
                        for q in range(0, vnch, 4):
                            k = min(4, vnch - q)
                            tps = ppool1.tile([P, 4, P], BF16, space="PSUM",
                                              tag="tps")
                            for j in range(k):
                                nc.tensor.transpose(
                                    out=tps[:, j, :],
                                    in_=m0_t[:, (q + j) * P:(q + j + 1) * P],
                                    identity=ident_bf[:])
                            nc.scalar.activation(
                                out=m0t_t[:, q * P:(q + k) * P],
                                in_=tps[:, 0:k, :].rearrange("p a b -> p (a b)"),
                                func=mybir.ActivationFunctionType.Copy)
                        w1i = nc.scalar.dma_start(out=m0t_d[gi, :, 0:vnch * P],
                                                  in_=m0t_t[:, 0:vnch * P])
                        m0w[gi] = (w0.ins, w1i.ins)
                    else:
                        m0_t = gpool.tile([P, VCHmax * P], BF16, tag="m0")
                        ld = nc.sync.dma_start(out=m0_t[:, 0:vnch * P],
                                               in_=m0_d[gi, :, 0:vnch * P])
                        add_dep_helper(ld.ins, m0w[gi][0], reason="m0 built")
                        m0t_t = gpool.tile([P, VCHmax * P], BF16, tag="m0t")
                        ld = nc.sync.dma_start(out=m0t_t[:, 0:vnch * P],
                                               in_=m0t_d[gi, :, 0:vnch * P])
                        add_dep_helper(ld.ins, m0w[gi][1], reason="m0t built")
                    # d-expansion: tp[pp, ph, h] = d[dslot(pp, ph), h],
                    # accumulated over the phys chunk's virtual columns
                    tp = ppool1.tile([P, CHmax, H], F32, space="PSUM", tag="tp")
                    for (vc, ph, st_abs, first, last) in g.tpl:
                        nc.tensor.matmul(
                            out=tp[:, ph, 0:heads],
                            lhsT=m0t_t[:, vc * P:(vc + 1) * P],
                            rhs=down[:, st_abs, 0:heads],
                            start=first, stop=last)
                    # whole-group softmax numerators: ex = exp(leakyrelu(s + d))
                    ts_t = wpool.tile([P, CHmax, H], F32, tag="ts")
                    ex_t = wpool.tile([P, CHmax, H], F32, tag="ex")
                    nc.vector.tensor_tensor(
                        out=ts_t[:, 0:nch, 0:heads],
                        in0=tp[:, 0:nch, 0:heads],
                        in1=gath[:, 0:nch, scol:scol + heads],
                        op=mybir.AluOpType.add)
                    # exp(leakyrelu(z)) == max(exp(z), exp(neg*z)): two scaled
                    # ACT exps + one DVE max keeps the slope exact (the HW
                    # Lrelu LUT ignores the alpha operand)
                    nc.scalar.activation(
                        out=ex_t[:, 0:nch, 0:heads],
                        in_=ts_t[:, 0:nch, 0:heads],
                        func=mybir.ActivationFunctionType.Exp)
                    nc.scalar.activation(
                        out=ts_t[:, 0:nch, 0:heads],
                        in_=ts_t[:, 0:nch, 0:heads],
                        func=mybir.ActivationFunctionType.Exp, scale=cfg.neg)
                    nc.vector.tensor_tensor(
                        out=ex_t[:, 0:nch, 0:heads],
                        in0=ex_t[:, 0:nch, 0:heads],
                        in1=ts_t[:, 0:nch, 0:heads],
                        op=mybir.AluOpType.max)
                    # scale messages (incl. ones-col -> denominator)
                    for h in range(heads):
                        nc.vector.tensor_tensor(
                            out=gath[:, 0:nch, h * mw:(h + 1) * mw],
                            in0=gath[:, 0:nch, h * mw:(h + 1) * mw],
                            in1=ex_t[:, 0:nch, h:h + 1].to_broadcast([P, nch, mw]),
                            op=mybir.AluOpType.mult)
                    # scatter-add into [dst, heads*mw] PSUM per supertile
                    aggp = ppool1.tile([P, cfg.GST, heads, mw], F32, space="PSUM",
                                       tag="agg")
                    for (vc, ph, sti, first, last) in g.aggl:
                        nc.tensor.matmul(
                            out=aggp[:, sti, :, :].rearrange("p h m -> p (h m)"),
                            lhsT=m0_t[:, vc * P:(vc + 1) * P],
                            rhs=gath[:, ph, 0:heads * mw],
                            start=first, stop=last)
                    # normalize whole group
                    # (layer-1 messages are [h|ones], layer-2 [ones|h])
                    dcol = mw - 1 if layer == 1 else 0
                    rec = wpool.tile([P, cfg.GST, heads, 1], F32, tag="rec")
                    # +eps: pad dst rows have zero denominators (no edges)
                    nc.vector.tensor_scalar_add(
                        out=rec[:, 0:L], in0=aggp[:, 0:L, :, dcol:dcol + 1],
                        scalar1=1e-30)
                    nc.vector.reciprocal(out=rec[:, 0:L], in_=rec[:, 0:L])
                    g0 = g.sts[0]
                    if layer == 1:
                        gv = g_all[:, g0:g0 + L, :, :]
                        nc.vector.tensor_tensor(
                            out=gv, in0=aggp[:, 0:L, :, 0:cfg.HID],
                            in1=rec[:, 0:L].to_broadcast([P, L, heads, cfg.HID]),
                            op=mybir.AluOpType.mult)
                        nc.vector.tensor_tensor(
                            out=gv, in0=gv,
                            in1=b1_bc[:].to_broadcast([P, L, H, cfg.HID]),
                            op=mybir.AluOpType.add)
                        gvf = gv.rearrange("p s h d -> p (s h d)")
                        nc.scalar.activation(
                            out=gvf, in_=gvf,
                            func=mybir.ActivationFunctionType.Gelu)
                        # interleaved table2 build for this group's supertiles
                        for st in g.sts:
                            gT_p = ppool2.tile([H * cfg.HID, P], BF16, space="PSUM",
                                               tag="gT")
                            nc.tensor.transpose(
                                out=gT_p[:],
                                in_=g_all[:, st, :, :].rearrange("p h d -> p (h d)"),
                                identity=ident_bf[:])
                            gT = wpool.tile([H * cfg.HID, P], F32, tag="gTs")
                            nc.scalar.activation(
                                out=gT[:], in_=gT_p[:],
                                func=mybir.ActivationFunctionType.Copy)
                            h2p = ppool2.tile([P, NC2], F32, space="PSUM", tag="h2p")
                            nc.tensor.matmul(out=h2p[:], lhsT=gT[:], rhs=w2e[:, :],
                                             start=True, stop=True)
                            # table-2 row: [ones | h2 | s | d]
                            pack = wpool.tile([P, P], BF16, tag="pack")
                            nc.vector.memset(pack[:, 0:1], 1.0)
                            nc.vector.tensor_copy(out=pack[:, 1:1 + NC2],
                                                  in_=h2p[:, 0:NC2])
                            nc.vector.tensor_copy(out=d2o[:, st, :],
                                                  in_=h2p[:, NC2 - 1:NC2])
                            t2_writes.append(nc.sync.dma_start(
                                out=t2_shard[st * P:(st + 1) * P, :],
                                in_=pack[:]).ins)
                    else:
                        ov = wpool.tile([P, cfg.GST, cfg.OUT], F32, tag="ov")
                        nc.vector.tensor_tensor(
                            out=ov[:, 0:L, :], in0=aggp[:, 0:L, 0, 1:1 + cfg.OUT],
                            in1=rec[:, 0:L, 0, :].to_broadcast([P, L, cfg.OUT]),
                            op=mybir.AluOpType.mult)
                        ovb = wpool.tile([P, cfg.GST, cfg.OUT], BF16, tag="ovb")
                        nc.vector.tensor_tensor(
                            out=ovb[:, 0:L, :], in0=ov[:, 0:L, :],
                            in1=b2_bc[:].to_broadcast([P, L, cfg.OUT]),
                            op=mybir.AluOpType.add)
                        for i, st_abs in enumerate(g.sts):
                            rows = min(P, cfg.SHARD - st_abs * P)
                            nc.sync.dma_start(
                                out=out_ext[st_abs * P:st_abs * P + rows, :]
                                    .bitcast(BF16),
                                in_=ovb[0:rows, i, :])
                if layer == 1:
                    _ppt.__exit__(None, None, None)
                _ppe.__exit__(None, None, None)

            t2_writes = []
            edge_pass(1)

            if cfg.no_cc:
                cc_inst = nc.sync.dma_start(out=t2_full[0:SP, :], in_=t2_shard[:])
            else:
                cc_inst = nc.gpsimd.collective_compute(
                    "AllGather", mybir.AluOpType.bypass,
                    ins=[t2_shard[:].opt()], outs=[t2_full[:].opt()],
                    replica_groups=[list(range(cfg.CORES))])
            for w in t2_writes:
                add_dep_helper(cc_inst.ins, w, reason="t2 shard complete before AG")
            fences[2] = cc_inst.ins

            edge_pass(2)
            _gpcm.__exit__(None, None, None)

    nc.compile()
    return nc


_CACHE = {}


def _get_built(cfg, edge_index):
    key = hash((edge_index.tobytes(), cfg.N, cfg.E, cfg.GST, cfg.sim_gelu,
                cfg.no_cc, cfg.QN, cfg.GBUF, cfg.WBUF))
    if key not in _CACHE:
        sched, arrays = preprocess(edge_index, cfg)
        nc = build_nc(cfg, sched)
        _CACHE[key] = (nc, sched, arrays)
    return _CACHE[key]


def make_in_maps(cfg, arrays, inputs):
    x = np.ascontiguousarray(inputs["x"], dtype=np.float32)
    shared = dict(
        W1=np.ascontiguousarray(inputs["W1"], dtype=np.float32),
        a_src1=np.ascontiguousarray(inputs["a_src1"], dtype=np.float32),
        a_dst1=np.ascontiguousarray(inputs["a_dst1"], dtype=np.float32),
        b1=np.ascontiguousarray(inputs["b1"], dtype=np.float32).reshape(1, -1),
        W2=np.ascontiguousarray(inputs["W2"], dtype=np.float32),
        a_src2=np.ascontiguousarray(inputs["a_src2"], dtype=np.float32),
        a_dst2=np.ascontiguousarray(inputs["a_dst2"], dtype=np.float32),
        b2=np.ascontiguousarray(inputs["b2"], dtype=np.float32).reshape(1, -1),
    )
    in_maps = []
    for r in range(cfg.CORES):
        xr = np.zeros((cfg.SHARD_PAD, cfg.IN), dtype=np.float32)
        xr[0:cfg.SHARD] = x[r * cfg.SHARD:(r + 1) * cfg.SHARD]
        m = dict(shared)
        m["xT"] = np.ascontiguousarray(
            xr.T.astype(ml_dtypes.bfloat16)).view(np.float32)
        m["idx"] = arrays[r]["idx"]
        m["dsl"] = arrays[r]["dsl"]
        in_maps.append(m)
    return in_maps


def kernel(x, edge_index, W1, a_src1, a_dst1, b1, W2, a_src2, a_dst2, b2,
           cfg=None, return_extras=False):
    from concourse.bass_utils import run_bass_kernel_spmd
    cfg = cfg or Cfg()
    nc, sched, arrays = _get_built(cfg, np.asarray(edge_index))
    in_maps = make_in_maps(cfg, arrays, dict(
        x=x, W1=W1, a_src1=a_src1, a_dst1=a_dst1, b1=b1,
        W2=W2, a_src2=a_src2, a_dst2=a_dst2, b2=b2))
    res = run_bass_kernel_spmd(nc, in_maps, list(range(cfg.CORES)))
    out = np.concatenate(
        [np.ascontiguousarray(res.results[r]["out"])
         .view(ml_dtypes.bfloat16).astype(np.float32)
         for r in range(cfg.CORES)],
        axis=0)
    if return_extras:
        return out, res
    return out


# revision 34
# speedup vs baseline: 1.0902x; 1.0191x over previous
"""Trainium2 Bass kernel for 2-layer GAT (nn_GAT_86535001080291) — v3.

Strategy (dst-sharded graph parallelism over 8 NeuronCores):
  - Inputs per core: x shard transposed bf16 [64, 12544] (~1.6MB), compact
    per-slot metadata (gather indices [NG,16,C16] int16 and dst-slot ids dsl
    [128, TOTVCH] u8), and the small weights.
  - Each core builds table1 for its OWN node shard from x, then AllGathers
    the full table (bf16 [100352, 128] rows) into Shared DRAM.
  - Edge gathers run on gpsimd dma_gather spread over 4 SWDGE queues (the
    descriptor-generation ucode pins one Q7 core pair per queue, so
    round-robining buckets over queues engages all 8 Q7 cores).
  - v3: the one-hot M0 ([slot,128dst] per 128-edge chunk) and its transpose
    are built INLINE in the layer-1 edge pass (bf16, straight in SBUF via a
    DVE is_equal against a repeating iota + PE transposes), used directly as
    matmul operands, and stored bf16 to DRAM only for layer 2's reload.
    The layer-1 AllGather is triggered straight after the table build, so
    edge gathers start as soon as the collective lands instead of waiting
    for a separate M0-build phase (and the per-group loads no longer queue
    behind 25 groups of M0 stores on the sync DMA queue — stores go out on
    the scalar queue instead).
  - Edge pass per group of GST supertiles: gpsimd.dma_gather of 256B source
    rows from the table, d-expansion via M0T matmuls, leakyrelu + exp on
    DVE/ACT over the whole group, message scaling, and scatter-add via M0
    matmuls accumulating in PSUM (ones-column = softmax denominator).
All host-side preprocessing depends only on edge_index (graph structure).
"""
import math
from dataclasses import dataclass, field

import numpy as np
import ml_dtypes

import concourse.bacc as bacc
import concourse.mybir as mybir
from concourse.tile import TileContext
from concourse.masks import make_identity
from concourse.tile_rust import add_dep_helper
from concourse import library_config

F32 = mybir.dt.float32
BF16 = mybir.dt.bfloat16
I16 = mybir.dt.int16
P = 128


@dataclass
class Cfg:
    N: int = 100000
    E: int = 1600000
    IN: int = 64
    HID: int = 32
    HEADS: int = 2
    OUT: int = 32
    neg: float = 0.2
    CORES: int = 8
    GST: int = 2            # supertiles per group
    GBUF: int = 4           # group-pipeline depth (gather-pool buffers)
    WBUF: int = 6           # work-pool depth
    QN: int = 4             # SWDGE queues for dma_gather round-robin
    sim_gelu: bool = False  # kept for compat; unused (HW Gelu LUT)
    no_cc: bool = False     # timing experiment: replace collectives w/ copies

    @property
    def SHARD(self):
        return self.N // self.CORES

    @property
    def ST(self):
        return math.ceil(self.SHARD / P)

    @property
    def SHARD_PAD(self):
        return self.ST * P

    @property
    def NT(self):
        return self.CORES * self.SHARD_PAD

    @property
    def BROWS(self):
        nb = self.NBUCK
        return (self.NT + nb - 1) // nb

    @property
    def NBUCK(self):
        return max(1, math.ceil(self.NT / 25088))


@dataclass
class GroupSched:
    sts: list            # absolute supertile ids
    nch: int = 0         # physical 128-slot chunks in group
    vnch: int = 0        # virtual chunks (one per (phys chunk, supertile) pair)
    c16: int = 0         # idx columns (slots/16)
    gch0: int = 0        # global phys chunk offset of this group
    gvc0: int = 0        # global virtual chunk offset of this group
    calls: list = field(default_factory=list)    # per bucket: (off16, nidx, ch0)
    cellstart: dict = field(default_factory=dict)  # (sti, b) -> slot offset
    cellvc0: dict = field(default_factory=dict)    # (sti, b) -> group-rel vc id
    tpl: list = field(default_factory=list)   # (vc, ph, st_abs, first, last) per phys
    aggl: list = field(default_factory=list)  # (vc, ph, sti, first, last) per st


def build_schedule(cfg, C_sb):
    """Shared (core-independent) static schedule.

    Cells (per supertile, bucket) are sized to the exact max-over-cores edge
    count; slots for a (group, bucket) run are pooled back-to-back and only
    the run is padded to 128 (gather output alignment). 128-slot physical
    chunks may straddle supertile boundaries; such chunks get one VIRTUAL
    one-hot column block per supertile present.
    """
    groups = []
    st = 0
    gch0 = 0
    gvc0 = 0
    while st < cfg.ST:
        sts = list(range(st, min(st + cfg.GST, cfg.ST)))
        g = GroupSched(sts=sts, gch0=gch0, gvc0=gvc0)
        slot = 0
        vcs = []  # (vc_rel, ph_rel, sti, st_abs)
        for b in range(cfg.NBUCK):
            off16 = slot // 16   # bucket runs start 128-aligned
            s0b = slot
            for i, s in enumerate(sts):
                size = int(C_sb[s][b])
                g.cellstart[(i, b)] = slot
                g.cellvc0[(i, b)] = len(vcs)
                if size > 0:
                    ph0 = slot // P
                    ph1 = (slot + size - 1) // P
                    for ph in range(ph0, ph1 + 1):
                        vcs.append((len(vcs), ph, i, s))
                slot += size
            slot = (slot + P - 1) // P * P
            g.calls.append((off16, slot - s0b, s0b // P))
        g.nch = slot // P
        g.c16 = slot // 16
        g.vnch = len(vcs)
        by_phys = {}
        for (vc, ph, i, s) in vcs:
            by_phys.setdefault(ph, []).append(vc)
        for (vc, ph, i, s) in vcs:
            lst = by_phys[ph]
            g.tpl.append((vc, ph, s, vc == lst[0], vc == lst[-1]))
        by_st = {}
        for (vc, ph, i, s) in vcs:
            by_st.setdefault(i, []).append((vc, ph))
        for i, s in enumerate(sts):
            lst = by_st.get(i, [])
            for k, (vc, ph) in enumerate(lst):
                g.aggl.append((vc, ph, i, k == 0, k == len(lst) - 1))
        groups.append(g)
        gch0 += g.nch
        gvc0 += g.vnch
        st += cfg.GST
    return groups


def preprocess(edge_index, cfg):
    """Pure graph preprocessing: per-core gather indices + dst-slot metadata."""
    src = edge_index[0].astype(np.int64)
    dst = edge_index[1].astype(np.int64)
    loops = np.arange(cfg.N, dtype=np.int64)
    src = np.concatenate([src, loops])
    dst = np.concatenate([dst, loops])

    SH, SP, ST, NB, BR = cfg.SHARD, cfg.SHARD_PAD, cfg.ST, cfg.NBUCK, cfg.BROWS

    per_core = []
    cnt = np.zeros((cfg.CORES, ST, NB), dtype=np.int64)
    SPH = SP // 2
    NTH = cfg.NT // 2
    for r in range(cfg.CORES):
        m = (dst >= r * SH) & (dst < (r + 1) * SH)
        s_r = src[m]
        d_r = dst[m] - r * SH
        # table rows are laid out in two half-shard slabs so each half can
        # be AllGathered independently (and layer-2 gathers for buckets 0-1
        # only wait on the first half-collective)
        l_r = s_r % SH
        h_r = (l_r >= SPH).astype(np.int64)
        srow = h_r * NTH + (s_r // SH) * SPH + (l_r - h_r * SPH)
        b_r = srow // BR
        st_r = d_r // P
        per_core.append((srow, d_r, b_r, st_r))
        np.add.at(cnt[r], (st_r, b_r), 1)

    C_sb = cnt.max(axis=0)  # [ST, NB] exact max-over-cores cell sizes
    groups = build_schedule(cfg, C_sb)
    NG = len(groups)
    CHmax = max(g.nch for g in groups)
    VCHmax = max(g.vnch for g in groups)
    C16max = max(g.c16 for g in groups)
    C16max += C16max % 2                      # even: idx ships as f32-typed
    TOTVCH = sum(g.vnch for g in groups)
    TOTVCH4 = (TOTVCH + 3) // 4 * 4           # /4: dsl ships as f32-typed

    # lookup tables per (st, b) cell
    gi_tab = np.zeros((ST, NB), np.int64)
    cstart_tab = np.zeros((ST, NB), np.int64)   # group-relative slot offset
    vc0_tab = np.zeros((ST, NB), np.int64)      # GLOBAL vc id of cell's first vc
    ph0_tab = np.zeros((ST, NB), np.int64)      # group-relative phys chunk of it
    for gi, g in enumerate(groups):
        for (i, b), s0 in g.cellstart.items():
            s = g.sts[i]
            gi_tab[s, b] = gi
            cstart_tab[s, b] = s0
            vc0_tab[s, b] = g.gvc0 + g.cellvc0[(i, b)]
            ph0_tab[s, b] = s0 // P

    arrays = []
    for r in range(cfg.CORES):
        srow, d_r, b_r, st_r = per_core[r]
        gi_r = st_r // cfg.GST
        # sort edges by (group, bucket, st), stable
        order = np.lexsort((st_r, b_r, gi_r))
        srow, d_r, b_r, st_r = srow[order], d_r[order], b_r[order], st_r[order]

        # rank within each (st, b) cell
        cell_key = st_r * NB + b_r
        change = np.empty(len(cell_key), dtype=bool)
        change[0] = True
        change[1:] = cell_key[1:] != cell_key[:-1]
        starts = np.flatnonzero(change)
        rank = np.arange(len(cell_key)) - np.repeat(
            starts, np.diff(np.append(starts, len(cell_key))))

        gi_e = gi_tab[st_r, b_r]
        slot = cstart_tab[st_r, b_r] + rank    # group-relative slot
        vcol = vc0_tab[st_r, b_r] + slot // P - ph0_tab[st_r, b_r]

        idx16 = np.zeros((NG, 16, C16max), dtype=np.int16)
        idx16[gi_e, slot % 16, slot // 16] = (srow - b_r * BR).astype(np.int16)

        dsl = np.full((P, TOTVCH4), 255, dtype=np.uint8)
        dsl[slot % P, vcol] = (d_r % P).astype(np.uint8)

        # ship as f32-typed buffers (same bytes) — the tunnel moves f32
        # noticeably faster than 8/16-bit dtypes
        arrays.append(dict(
            idx=np.ascontiguousarray(idx16).view(np.float32),
            dsl=np.ascontiguousarray(dsl).view(np.float32),
        ))

    sched = dict(groups=groups, NG=NG, CHmax=CHmax, VCHmax=VCHmax,
                 C16max=C16max, TOTVCH=TOTVCH, TOTVCH4=TOTVCH4)
    return sched, arrays


def build_nc(cfg, sched):
    nc = bacc.Bacc("TRN2", target_bir_lowering=False,
                   num_swdge_queues=cfg.QN)
    NG, CHmax, C16max = sched["NG"], sched["CHmax"], sched["C16max"]
    VCHmax, TOTVCH, TOTVCH4 = sched["VCHmax"], sched["TOTVCH"], sched["TOTVCH4"]
    groups = sched["groups"]
    ST, NT, SP, NB, BR = cfg.ST, cfg.NT, cfg.SHARD_PAD, cfg.NBUCK, cfg.BROWS
    H = cfg.HEADS

    # ---- external I/O ----
    # All bulk I/O is declared f32-typed (same bytes, device-side bitcast):
    # the host<->device tunnel ships f32 buffers measurably faster.
    xT_ext = nc.dram_tensor("xT", [cfg.IN, SP // 2], F32, kind="ExternalInput")
    W1_ext = nc.dram_tensor("W1", [cfg.IN, H * cfg.HID], F32, kind="ExternalInput")
    as1_ext = nc.dram_tensor("a_src1", [H, cfg.HID], F32, kind="ExternalInput")
    ad1_ext = nc.dram_tensor("a_dst1", [H, cfg.HID], F32, kind="ExternalInput")
    b1_ext = nc.dram_tensor("b1", [1, H * cfg.HID], F32, kind="ExternalInput")
    W2_ext = nc.dram_tensor("W2", [H * cfg.HID, cfg.OUT], F32, kind="ExternalInput")
    as2_ext = nc.dram_tensor("a_src2", [1, cfg.OUT], F32, kind="ExternalInput")
    ad2_ext = nc.dram_tensor("a_dst2", [1, cfg.OUT], F32, kind="ExternalInput")
    b2_ext = nc.dram_tensor("b2", [1, cfg.OUT], F32, kind="ExternalInput")
    idx_ext = nc.dram_tensor("idx", [NG, 16, C16max // 2], F32, kind="ExternalInput")
    dsl_ext = nc.dram_tensor("dsl", [P, TOTVCH4 // 4], F32, kind="ExternalInput")
    out_ext = nc.dram_tensor("out", [cfg.SHARD, cfg.OUT // 2], F32,
                             kind="ExternalOutput")

    with TileContext(nc) as tc:
        with (
            tc.tile_pool(name="dram", bufs=1, space="DRAM") as dpool,
            tc.tile_pool(name="const", bufs=1) as cpool,
            tc.tile_pool(name="work", bufs=cfg.WBUF) as wpool,
        ):
            nc.gpsimd.load_library(library_config.mlp)

            t1_shard = dpool.tile([SP, P], BF16)
            t1_full_a = dpool.tile([NT // 2, P], BF16, addr_space="Shared")
            t1_full_b = dpool.tile([NT // 2, P], BF16, addr_space="Shared")
            t1_full = [t1_full_a, t1_full_b]
            t2_shard = dpool.tile([SP, P], BF16)
            t2_full_a = dpool.tile([NT // 2, P], BF16, addr_space="Shared")
            t2_full_b = dpool.tile([NT // 2, P], BF16, addr_space="Shared")
            t2_full = [t2_full_a, t2_full_b]
            m0_d = dpool.tile([NG, P, VCHmax * P], BF16)
            m0t_d = dpool.tile([NG, P, VCHmax * P], BF16)
            idxr_d = dpool.tile([NG, P, C16max], I16)

            ident = cpool.tile([P, P], F32)
            make_identity(nc, ident[:])
            ident_bf = cpool.tile([P, P], BF16)
            make_identity(nc, ident_bf[:])
            _pp0cm = tc.tile_pool(name="psum0", bufs=2, space="PSUM")
            ppool0 = _pp0cm.__enter__()

            # repeating [0..127] iota, compared against dsl to build one-hots
            iota_rep = cpool.tile([P, VCHmax, P], BF16)
            nc.gpsimd.iota(iota_rep[:], pattern=[[0, VCHmax], [1, P]],
                           base=0, channel_multiplier=0,
                           allow_small_or_imprecise_dtypes=True)

            # ---------- idx replication to 128 partitions (DRAM->DRAM) ----------
            idxr_writes = []
            for r in range(8):
                idxr_writes.append(nc.sync.dma_start(
                    out=idxr_d[:, 16 * r:16 * (r + 1), :],
                    in_=idx_ext[:, :, :].bitcast(I16)).ins)

            # ---------- weight prep ----------
            w1_t = cpool.tile([cfg.IN, H * cfg.HID], F32)
            nc.sync.dma_start(out=w1_t[:], in_=W1_ext[:, :])
            w2_t = cpool.tile([H * cfg.HID, cfg.OUT], F32)
            nc.sync.dma_start(out=w2_t[:], in_=W2_ext[:, :])
            # a vectors as [HID, 1] columns
            av = cpool.tile([cfg.HID, 2 * H + 2], F32)
            for h in range(H):
                nc.sync.dma_start(out=av[:, h:h + 1], in_=as1_ext[h:h + 1, :])
                nc.sync.dma_start(out=av[:, H + h:H + h + 1], in_=ad1_ext[h:h + 1, :])
            nc.sync.dma_start(out=av[:, 2 * H:2 * H + 1], in_=as2_ext[0:1, :])
            nc.sync.dma_start(out=av[:, 2 * H + 1:2 * H + 2], in_=ad2_ext[0:1, :])

            # per-head W1 transposes (base partition 0)
            w1Th = cpool.tile([cfg.HID, H, cfg.IN], F32)
            for h in range(H):
                w1Th_p = ppool0.tile([cfg.HID, cfg.IN], F32, space="PSUM", tag="prep")
                nc.tensor.transpose(out=w1Th_p[:],
                                    in_=w1_t[:, h * cfg.HID:(h + 1) * cfg.HID],
                                    identity=ident[0:cfg.IN, 0:cfg.IN])
                nc.vector.tensor_copy(out=w1Th[:, h, :], in_=w1Th_p[:])
            w2T_p = ppool0.tile([cfg.OUT, H * cfg.HID], F32, space="PSUM", tag="prep")
            nc.tensor.transpose(out=w2T_p[:], in_=w2_t[:, :],
                                identity=ident[0:H * cfg.HID, 0:H * cfg.HID])
            w2T = cpool.tile([cfg.OUT, H * cfg.HID], F32)
            nc.vector.tensor_copy(out=w2T[:], in_=w2T_p[:])

            # logit weight vectors: wv1[:, 0:2H] = per-head [src..., dst...]
            wv_p = ppool0.tile([cfg.IN, 2 * H + 2], F32, space="PSUM", tag="prep2")
            for h in range(H):
                nc.tensor.matmul(out=wv_p[:, h:h + 1],
                                 lhsT=w1Th[:, h, :],
                                 rhs=av[0:cfg.HID, h:h + 1], start=True, stop=True)
                nc.tensor.matmul(out=wv_p[:, H + h:H + h + 1],
                                 lhsT=w1Th[:, h, :],
                                 rhs=av[0:cfg.HID, H + h:H + h + 1], start=True, stop=True)
            # layer2 vectors: W2 @ a_src2 : contraction over OUT
            nc.tensor.matmul(out=wv_p[0:H * cfg.HID, 2 * H:2 * H + 1], lhsT=w2T[:, :],
                             rhs=av[0:cfg.OUT, 2 * H:2 * H + 1], start=True, stop=True)
            nc.tensor.matmul(out=wv_p[0:H * cfg.HID, 2 * H + 1:2 * H + 2], lhsT=w2T[:, :],
                             rhs=av[0:cfg.OUT, 2 * H + 1:2 * H + 2], start=True, stop=True)

            # W1ext bf16 [IN, 70]: [W1h0 | 0 | W1h1 | 0 | s0 s1 d0 d1]
            NC1 = 2 * (cfg.HID + 1) + 2 * H
            SD1 = 2 * (cfg.HID + 1)  # offset of s-cols in table1
            w1e = cpool.tile([cfg.IN, NC1], BF16)
            for h in range(H):
                nc.vector.tensor_copy(out=w1e[:, h * (cfg.HID + 1):h * (cfg.HID + 1) + cfg.HID],
                                      in_=w1_t[:, h * cfg.HID:(h + 1) * cfg.HID])
                nc.vector.memset(w1e[:, h * (cfg.HID + 1) + cfg.HID:(h + 1) * (cfg.HID + 1)], 0.0)
            nc.vector.tensor_copy(out=w1e[:, SD1:SD1 + H], in_=wv_p[:, 0:H])
            nc.vector.tensor_copy(out=w1e[:, SD1 + H:NC1], in_=wv_p[:, H:2 * H])
            # W2ext f32 [64, 34]: [W2 | s2vec | d2vec]
            NC2 = cfg.OUT + 2
            w2e = cpool.tile([H * cfg.HID, NC2], F32)
            nc.vector.tensor_copy(out=w2e[:, 0:cfg.OUT], in_=w2_t[:, :])
            nc.vector.tensor_copy(out=w2e[:, cfg.OUT:NC2],
                                  in_=wv_p[0:H * cfg.HID, 2 * H:2 * H + 2])

            # biases broadcast to all partitions, with a leading unit axis for
            # per-group (GST-wide) broadcasts
            b1_bc = cpool.tile([P, 1, H, cfg.HID], F32)
            b1_row = cpool.tile([1, H * cfg.HID], F32)
            nc.sync.dma_start(out=b1_row[:], in_=b1_ext[:, :])
            nc.gpsimd.partition_broadcast(
                out_ap=b1_bc[:].rearrange("p a h d -> p (a h d)"), in_ap=b1_row[:])
            b2_bc = cpool.tile([P, 1, cfg.OUT], F32)
            b2_row = cpool.tile([1, cfg.OUT], F32)
            nc.sync.dma_start(out=b2_row[:], in_=b2_ext[:, :])
            nc.gpsimd.partition_broadcast(
                out_ap=b2_bc[:].rearrange("p a d -> p (a d)"), in_ap=b2_row[:])

            d1o = cpool.tile([P, ST, H], BF16)
            d2o = cpool.tile([P, ST, 1], BF16)
            g_all = cpool.tile([P, ST, H, cfg.HID], BF16)

            # ---------- phase T1: own-shard table1 build ----------
            t1_writes = {}
            for st in range(ST):
                xTt = wpool.tile([cfg.IN, P], BF16, tag="xT")
                nc.sync.dma_start(
                    out=xTt[:],
                    in_=xT_ext[:, st * (P // 2):(st + 1) * (P // 2)].bitcast(BF16))
                hp = ppool0.tile([P, NC1], F32, space="PSUM", tag="hp")
                nc.tensor.matmul(out=hp[:, :], lhsT=xTt[:], rhs=w1e[:, :],
                                 start=True, stop=True)
                pack = wpool.tile([P, P], BF16, tag="pack")
                nc.vector.tensor_copy(out=pack[:, 0:NC1], in_=hp[:, :])
                ones_view = pack[:, 0:SD1].rearrange(
                    "p (h d) -> p h d", h=H)[:, :, cfg.HID:cfg.HID + 1]
                nc.vector.memset(ones_view, 1.0)
                nc.vector.tensor_copy(out=d1o[:, st, :], in_=hp[:, SD1 + H:SD1 + 2 * H])
                t1_writes[st] = nc.sync.dma_start(
                    out=t1_shard[st * P:(st + 1) * P, :], in_=pack[:]).ins

            _pp0cm.__exit__(None, None, None)

            # Each table AllGather is split into two half-shard collectives
            # (the row remap in preprocess puts buckets 0-1 entirely in half
            # 0 and buckets 2-3 in half 1), so bucket gathers only wait on
            # their own half and the layer-2 first half fires mid-layer-1.
            SPH = SP // 2
            NTH = NT // 2
            HSTS = SPH // P

            def half_cc(shard, full, h, writes):
                lo = h * SPH
                if cfg.no_cc:
                    cc = nc.sync.dma_start(
                        out=full[h][0:SPH, :],
                        in_=shard[lo:lo + SPH, :])
                else:
                    cc = nc.gpsimd.collective_compute(
                        "AllGather", mybir.AluOpType.bypass,
                        ins=[shard[lo:lo + SPH, :].opt()],
                        outs=[full[h][:, :].opt()],
                        replica_groups=[list(range(cfg.CORES))])
                for w in writes:
                    add_dep_helper(cc.ins, w, reason=f"table half {h} built")
                return cc.ins

            fences = {1: (
                half_cc(t1_shard, t1_full, 0,
                        [w for st, w in t1_writes.items() if st < HSTS]),
                half_cc(t1_shard, t1_full, 1,
                        [w for st, w in t1_writes.items() if st >= HSTS]),
            )}
            t2cc = [None, None]

            _gpcm = tc.tile_pool(name="gath", bufs=cfg.GBUF)
            gpool = _gpcm.__enter__()
            m0w = {}

            # ---------- shared edge-pass ----------
            def edge_pass(layer):
                # Per-layer PSUM pools: layer 1 needs tps/gT/h2p banks too, so
                # tp/agg stay at depth 2; layer 2 only needs tp/agg and gets
                # depth 4 (8 banks total either way).
                if layer == 1:
                    table, heads, scol = t1_full, H, SD1
                    mw = cfg.HID + 1   # per-head message width (h | ones)
                    down = d1o
                    edepth = 2
                else:
                    table, heads, scol = t2_full, 1, cfg.OUT + 1
                    mw = cfg.OUT + 1
                    down = d2o
                    edepth = 4
                _ppe = tc.tile_pool(name=f"psum_e{layer}", bufs=edepth,
                                    space="PSUM")
                ppool1 = _ppe.__enter__()
                if layer == 1:
                    _ppt = tc.tile_pool(name="psum_t", bufs=1, space="PSUM")
                    ppool2 = _ppt.__enter__()
                for gi, g in enumerate(groups):
                    nch = g.nch
                    vnch = g.vnch
                    L = len(g.sts)
                    # gathers first: independent of the M0 build, they only
                    # need the idx slice and the table fence
                    idx_t = gpool.tile([P, C16max], I16, tag="idx")
                    ld = nc.sync.dma_start(out=idx_t[:, 0:g.c16],
                                           in_=idxr_d[gi, :, 0:g.c16])
                    for w in idxr_writes:
                        add_dep_helper(ld.ins, w, reason="idx replicated")
                    gath = gpool.tile([P, CHmax, P], BF16, tag="gath")
                    for b in range(NB):
                        off16, nidx, ch0 = g.calls[b]
                        while nidx > 0:
                            n = min(nidx, 4096)
                            gi_inst = nc.gpsimd.dma_gather(
                                gath[:, ch0:ch0 + n // P, :],
                                table[b // 2][(b % 2) * BR:NTH, :],
                                idx_t[:, off16:off16 + n // 16],
                                n, n, P, single_packet=False,
                                queue_num=b % cfg.QN)
                            add_dep_helper(gi_inst.ins, fences[layer][b // 2],
                                           reason="table half ready")
                            nidx -= n
                            ch0 += n // P
                            off16 += n // 16
                    if layer == 1:
                        # M0 / M0T built inline in SBUF (bf16), stored to
                        # DRAM (scalar DMA queue) only for layer-2 reload
                        dslt8 = gpool.tile([P, VCHmax], mybir.dt.uint8, tag="dsl8")
                        nc.sync.dma_start(
                            out=dslt8[:, 0:vnch],
                            in_=dsl_ext[:, :].bitcast(mybir.dt.uint8)
                                [:, g.gvc0:g.gvc0 + vnch])
                        dslt = gpool.tile([P, VCHmax], BF16, tag="dsl")
                        nc.scalar.activation(
                            out=dslt[:, 0:vnch], in_=dslt8[:, 0:vnch],
                            func=mybir.ActivationFunctionType.Copy)
                        m0_t = gpool.tile([P, VCHmax * P], BF16, tag="m0")
                        nc.vector.tensor_tensor(
                            out=m0_t[:, 0:vnch * P].rearrange(
                                "p (a b) -> p a b", b=P),
                            in0=iota_rep[:, 0:vnch, :],
                            in1=dslt[:, 0:vnch].rearrange("p (a b) -> p a b", b=1)
                                .to_broadcast([P, vnch, P]),
                            op=mybir.AluOpType.is_equal)
                        w0 = nc.scalar.dma_start(out=m0_d[gi, :, 0:vnch * P],
                                                 in_=m0_t[:, 0:vnch * P])
                        m0t_t = gpool.tile([P, VCHmax * P], BF16, tag="m0t")
                        for q in range(0, vnch, 4):
                            k = min(4, vnch - q)
                            tps = ppool1.tile([P, 4, P], BF16, space="PSUM",
                                              tag="tps")
                            for j in range(k):
                                nc.tensor.transpose(
                                    out=tps[:, j, :],
                                    in_=m0_t[:, (q + j) * P:(q + j + 1) * P],
                                    identity=ident_bf[:])
                            nc.scalar.activation(
                                out=m0t_t[:, q * P:(q + k) * P],
                                in_=tps[:, 0:k, :].rearrange("p a b -> p (a b)"),
                                func=mybir.ActivationFunctionType.Copy)
                        w1i = nc.scalar.dma_start(out=m0t_d[gi, :, 0:vnch * P],
                                                  in_=m0t_t[:, 0:vnch * P])
                        m0w[gi] = (w0.ins, w1i.ins)
                    else:
                        m0_t = gpool.tile([P, VCHmax * P], BF16, tag="m0")
                        ld = nc.sync.dma_start(out=m0_t[:, 0:vnch * P],
                                               in_=m0_d[gi, :, 0:vnch * P])
                        add_dep_helper(ld.ins, m0w[gi][0], reason="m0 built")
                        m0t_t = gpool.tile([P, VCHmax * P], BF16, tag="m0t")
                        ld = nc.sync.dma_start(out=m0t_t[:, 0:vnch * P],
                                               in_=m0t_d[gi, :, 0:vnch * P])
                        add_dep_helper(ld.ins, m0w[gi][1], reason="m0t built")
                    # d-expansion: tp[pp, ph, h] = d[dslot(pp, ph), h],
                    # accumulated over the phys chunk's virtual columns
                    tp = ppool1.tile([P, CHmax, H], F32, space="PSUM", tag="tp")
                    for (vc, ph, st_abs, first, last) in g.tpl:
                        nc.tensor.matmul(
                            out=tp[:, ph, 0:heads],
                            lhsT=m0t_t[:, vc * P:(vc + 1) * P],
                            rhs=down[:, st_abs, 0:heads],
                            start=first, stop=last)
                    # whole-group softmax numerators: ex = exp(leakyrelu(s + d))
                    ts_t = wpool.tile([P, CHmax, H], F32, tag="ts")
                    ex_t = wpool.tile([P, CHmax, H], F32, tag="ex")
                    nc.vector.tensor_tensor(
                        out=ts_t[:, 0:nch, 0:heads],
                        in0=tp[:, 0:nch, 0:heads],
                        in1=gath[:, 0:nch, scol:scol + heads],
                        op=mybir.AluOpType.add)
                    # exp(leakyrelu(z)) == max(exp(z), exp(neg*z)): two scaled
                    # ACT exps + one DVE max keeps the slope exact (the HW
                    # Lrelu LUT ignores the alpha operand)
                    nc.scalar.activation(
                        out=ex_t[:, 0:nch, 0:heads],
                        in_=ts_t[:, 0:nch, 0:heads],
                        func=mybir.ActivationFunctionType.Exp)
                    nc.scalar.activation(
                        out=ts_t[:, 0:nch, 0:heads],
                        in_=ts_t[:, 0:nch, 0:heads],
                        func=mybir.ActivationFunctionType.Exp, scale=cfg.neg)
                    nc.vector.tensor_tensor(
                        out=ex_t[:, 0:nch, 0:heads],
                        in0=ex_t[:, 0:nch, 0:heads],
                        in1=ts_t[:, 0:nch, 0:heads],
                        op=mybir.AluOpType.max)
                    # scale messages (incl. ones-col -> denominator)
                    for h in range(heads):
                        nc.vector.tensor_tensor(
                            out=gath[:, 0:nch, h * mw:(h + 1) * mw],
                            in0=gath[:, 0:nch, h * mw:(h + 1) * mw],
                            in1=ex_t[:, 0:nch, h:h + 1].to_broadcast([P, nch, mw]),
                            op=mybir.AluOpType.mult)
                    # scatter-add into [dst, heads*mw] PSUM per supertile
                    aggp = ppool1.tile([P, cfg.GST, heads, mw], F32, space="PSUM",
                                       tag="agg")
                    for (vc, ph, sti, first, last) in g.aggl:
                        nc.tensor.matmul(
                            out=aggp[:, sti, :, :].rearrange("p h m -> p (h m)"),
                            lhsT=m0_t[:, vc * P:(vc + 1) * P],
                            rhs=gath[:, ph, 0:heads * mw],
                            start=first, stop=last)
                    # normalize whole group
                    # (layer-1 messages are [h|ones], layer-2 [ones|h])
                    dcol = mw - 1 if layer == 1 else 0
                    rec = wpool.tile([P, cfg.GST, heads, 1], F32, tag="rec")
                    # +eps: pad dst rows have zero denominators (no edges)
                    nc.vector.tensor_scalar_add(
                        out=rec[:, 0:L], in0=aggp[:, 0:L, :, dcol:dcol + 1],
                        scalar1=1e-30)
                    nc.vector.reciprocal(out=rec[:, 0:L], in_=rec[:, 0:L])
                    g0 = g.sts[0]
                    if layer == 1:
                        gv = g_all[:, g0:g0 + L, :, :]
                        nc.vector.tensor_tensor(
                            out=gv, in0=aggp[:, 0:L, :, 0:cfg.HID],
                            in1=rec[:, 0:L].to_broadcast([P, L, heads, cfg.HID]),
                            op=mybir.AluOpType.mult)
                        nc.vector.tensor_tensor(
                            out=gv, in0=gv,
                            in1=b1_bc[:].to_broadcast([P, L, H, cfg.HID]),
                            op=mybir.AluOpType.add)
                        gvf = gv.rearrange("p s h d -> p (s h d)")
                        nc.scalar.activation(
                            out=gvf, in_=gvf,
                            func=mybir.ActivationFunctionType.Gelu)
                        # interleaved table2 build for this group's supertiles
                        for st in g.sts:
                            gT_p = ppool2.tile([H * cfg.HID, P], BF16, space="PSUM",
                                               tag="gT")
                            nc.tensor.transpose(
                                out=gT_p[:],
                                in_=g_all[:, st, :, :].rearrange("p h d -> p (h d)"),
                                identity=ident_bf[:])
                            gT = wpool.tile([H * cfg.HID, P], F32, tag="gTs")
                            nc.scalar.activation(
                                out=gT[:], in_=gT_p[:],
                                func=mybir.ActivationFunctionType.Copy)
                            h2p = ppool2.tile([P, NC2], F32, space="PSUM", tag="h2p")
                            nc.tensor.matmul(out=h2p[:], lhsT=gT[:], rhs=w2e[:, :],
                                             start=True, stop=True)
                            # table-2 row: [ones | h2 | s | d]
                            pack = wpool.tile([P, P], BF16, tag="pack")
                            nc.vector.memset(pack[:, 0:1], 1.0)
                            nc.vector.tensor_copy(out=pack[:, 1:1 + NC2],
                                                  in_=h2p[:, 0:NC2])
                            nc.vector.tensor_copy(out=d2o[:, st, :],
                                                  in_=h2p[:, NC2 - 1:NC2])
                            t2_writes[st] = nc.sync.dma_start(
                                out=t2_shard[st * P:(st + 1) * P, :],
                                in_=pack[:]).ins
                        if gi == (HSTS - 1) // cfg.GST:
                            # first-half table-2 collective fires mid-pass
                            t2cc[0] = half_cc(
                                t2_shard, t2_full, 0,
                                [w for st, w in t2_writes.items() if st < HSTS])
                    else:
                        ov = wpool.tile([P, cfg.GST, cfg.OUT], F32, tag="ov")
                        nc.vector.tensor_tensor(
                            out=ov[:, 0:L, :], in0=aggp[:, 0:L, 0, 1:1 + cfg.OUT],
                            in1=rec[:, 0:L, 0, :].to_broadcast([P, L, cfg.OUT]),
                            op=mybir.AluOpType.mult)
                        ovb = wpool.tile([P, cfg.GST, cfg.OUT], BF16, tag="ovb")
                        nc.vector.tensor_tensor(
                            out=ovb[:, 0:L, :], in0=ov[:, 0:L, :],
                            in1=b2_bc[:].to_broadcast([P, L, cfg.OUT]),
                            op=mybir.AluOpType.add)
                        for i, st_abs in enumerate(g.sts):
                            rows = min(P, cfg.SHARD - st_abs * P)
                            nc.sync.dma_start(
                                out=out_ext[st_abs * P:st_abs * P + rows, :]
                                    .bitcast(BF16),
                                in_=ovb[0:rows, i, :])
                if layer == 1:
                    _ppt.__exit__(None, None, None)
                _ppe.__exit__(None, None, None)

            t2_writes = {}
            edge_pass(1)

            t2cc[1] = half_cc(
                t2_shard, t2_full, 1,
                [w for st, w in t2_writes.items() if st >= HSTS])
            fences[2] = tuple(t2cc)

            edge_pass(2)
            _gpcm.__exit__(None, None, None)

    nc.compile()
    return nc


_CACHE = {}


def _get_built(cfg, edge_index):
    key = hash((edge_index.tobytes(), cfg.N, cfg.E, cfg.GST, cfg.sim_gelu,
                cfg.no_cc, cfg.QN, cfg.GBUF, cfg.WBUF))
    if key not in _CACHE:
        sched, arrays = preprocess(edge_index, cfg)
        nc = build_nc(cfg, sched)
        _CACHE[key] = (nc, sched, arrays)
    return _CACHE[key]


def make_in_maps(cfg, arrays, inputs):
    x = np.ascontiguousarray(inputs["x"], dtype=np.float32)
    shared = dict(
        W1=np.ascontiguousarray(inputs["W1"], dtype=np.float32),
        a_src1=np.ascontiguousarray(inputs["a_src1"], dtype=np.float32),
        a_dst1=np.ascontiguousarray(inputs["a_dst1"], dtype=np.float32),
        b1=np.ascontiguousarray(inputs["b1"], dtype=np.float32).reshape(1, -1),
        W2=np.ascontiguousarray(inputs["W2"], dtype=np.float32),
        a_src2=np.ascontiguousarray(inputs["a_src2"], dtype=np.float32),
        a_dst2=np.ascontiguousarray(inputs["a_dst2"], dtype=np.float32),
        b2=np.ascontiguousarray(inputs["b2"], dtype=np.float32).reshape(1, -1),
    )
    in_maps = []
    for r in range(cfg.CORES):
        xr = np.zeros((cfg.SHARD_PAD, cfg.IN), dtype=np.float32)
        xr[0:cfg.SHARD] = x[r * cfg.SHARD:(r + 1) * cfg.SHARD]
        m = dict(shared)
        m["xT"] = np.ascontiguousarray(
            xr.T.astype(ml_dtypes.bfloat16)).view(np.float32)
        m["idx"] = arrays[r]["idx"]
        m["dsl"] = arrays[r]["dsl"]
        in_maps.append(m)
    return in_maps


def kernel(x, edge_index, W1, a_src1, a_dst1, b1, W2, a_src2, a_dst2, b2,
           cfg=None, return_extras=False):
    from concourse.bass_utils import run_bass_kernel_spmd
    cfg = cfg or Cfg()
    nc, sched, arrays = _get_built(cfg, np.asarray(edge_index))
    in_maps = make_in_maps(cfg, arrays, dict(
        x=x, W1=W1, a_src1=a_src1, a_dst1=a_dst1, b1=b1,
        W2=W2, a_src2=a_src2, a_dst2=a_dst2, b2=b2))
    res = run_bass_kernel_spmd(nc, in_maps, list(range(cfg.CORES)))
    out = np.concatenate(
        [np.ascontiguousarray(res.results[r]["out"])
         .view(ml_dtypes.bfloat16).astype(np.float32)
         for r in range(cfg.CORES)],
        axis=0)
    if return_extras:
        return out, res
    return out
